# revision 13
# baseline (speedup 1.0000x reference)
"""Trainium2 Bass kernel for the quirky MultiHeadAttention module.

Reference computation (B=4, S=1024, H=768, NH=12, HS=64):
    Q = (x@Wq+bq)  split into heads     [B,12,S,64]
    K = (x@Wk+bk)  split into heads     [B,12,S,64]
    V = x@Wv+bv    NOT split            [B,S,768]
    A = softmax(QK^T/8 + mask)          [B,12,S,S]
    out = (A @ V) reshaped [B, S*12, H] @ Wo + bo    -> [4, 12288, 768]

Algebraic restructuring:
  * (A @ V) @ Wo = A @ (x @ (Wv@Wo)) (+ cvec = bv@Wo + bo, realized by a
    constant row added to VW -- softmax rows sum to one).
  * Masked keys produce exp(-1e9+s) == 0 exactly, so they are dropped on
    the host and the key axis is compacted per batch.
  * The softmax denominator comes from a ones-column appended to VW.
    The device emits UNNORMALIZED numerators + the sigma column in f16;
    the host performs the division (allows partial-sum sharding).

Sharding: 8 cores = 4 batches x 2 head-groups (6 heads each), pure SPMD.
Key tiles are 128 wide. The base program uses B_kt tiles per core; when
exactly one "tile level" of overflow exists (e.g. per-batch tile counts
[4,4,4,5]), the overflow (batch, head-pair, tile) units are farmed out
one-per-core as an "extra unit" (cores of the overflowing batch run a
fully-masked dummy), and the host sums the unnormalized partials. This
removes the padding waste of a uniform max-tile program.

Layouts (all transposed so no on-device transposes are needed):
    QT/KT: [feat, tok] with head pairs packed 64+64 in partitions; the
        64-row score matmuls run 2-head-concurrent via PE row groups.
    S^T = KT.T @ QT -> [k, q]  (k on partitions => mask is a per-partition
        bias folded into the Exp activation)
    U = exp(S^T) [k, q] f16 -> exactly the layout the PV matmul needs
    out = U.T @ [VW | 1] -> [q, 769] with col 768 = sigma, stored f16.
Matmul operands are f16 (same PE speed as bf16, ~4x less rounding error
than bf16); accumulation is f32 in PSUM. Inputs are packed host-side into
partition-major [128, N] blobs and streamed over all three DMA queues
(sync/scalar HWDGE + gpsimd SWDGE) in consumption order; a burst of tiny
matmuls pre-warms the PE clock gate (HAM) to 2.4 GHz before real work.
"""

import math

import numpy as np

B, S, H, NH, HS = 4, 1024, 768, 12, 64
GW = 384          # head-group width = 6 heads * 64
NCORES = 8

_PROGRAM_CACHE = {}


def _pack6(a):
    """[768, N] -> partition-major [128, 6*N] (tile i at cols i*N:(i+1)*N)."""
    n = a.shape[1]
    return np.ascontiguousarray(
        a.reshape(6, 128, n).transpose(1, 0, 2).reshape(128, 6 * n))



def _build_program(bkt, has_extra, has_cvec):
    """bkt: number of base 128-wide key tiles per core (1..8).
    has_extra: include one (pair, tile) overflow unit per core.
    has_cvec: include the rank-1 (bv@Wo + bo) constant row in VW."""
    import concourse.mybir as mybir
    import concourse.tile as tile
    from concourse import bacc
    from concourse.bass import ds, ts

    f32 = mybir.dt.float32
    f16 = mybir.dt.float16
    AF = mybir.ActivationFunctionType

    KMAX = 128 * bkt
    if KMAX <= 512:
        kchunks = [(0, KMAX)]
    else:
        w1 = 128 * ((bkt + 1) // 2)
        kchunks = [(0, w1), (w1, KMAX - w1)]
    # sv columns: bq(3) bk(3) mk(bkt) [mke bqe bke]
    nsv = 6 + bkt + (3 if has_extra else 0)

    nc = bacc.Bacc(None, target_bir_lowering=False, debug=False)

    xp_d = nc.dram_tensor("xp", (128, 6 * 1024), f16, kind="ExternalInput")
    wqp_d = nc.dram_tensor("wqp", (128, 6 * 384), f16, kind="ExternalInput")
    wkp_d = nc.dram_tensor("wkp", (128, 6 * 384), f16, kind="ExternalInput")
    wvp_d = nc.dram_tensor("wvp", (128, 6 * 768), f16, kind="ExternalInput")
    sv_d = nc.dram_tensor("sv", (128, nsv), f32, kind="ExternalInput")
    if has_cvec:
        wvo6_d = nc.dram_tensor("wvo6", (1, 768), f16, kind="ExternalInput")
    if has_extra:
        xe_d = nc.dram_tensor("xe", (128, 6 * 1024), f16, kind="ExternalInput")
        xekt_d = nc.dram_tensor("xekt", (128, 6 * 128), f16,
                                kind="ExternalInput")
        wqe_d = nc.dram_tensor("wqe", (128, 6 * 128), f16,
                               kind="ExternalInput")
        wke_d = nc.dram_tensor("wke", (128, 6 * 128), f16,
                               kind="ExternalInput")
    # layout [j, hh, qc, p, mq, f]: DMA src iterates (partition, mq-block,
    # f), so those must be the three minor dims of the destination
    out_d = nc.dram_tensor("out", (3, 2, 2, 128, 4, 770), f16,
                           kind="ExternalOutput")
    if has_extra:
        oute_d = nc.dram_tensor("oute", (2, 2, 128, 4, 770), f16,
                                kind="ExternalOutput")

    with tile.TileContext(nc) as tc:
        with (
            tc.tile_pool(name="persist", bufs=1) as pp,
            tc.tile_pool(name="ut", bufs=4 * max(bkt, 2)) as utp,
            tc.tile_pool(name="osb", bufs=4) as op_,
        ):
            # ---- stream inputs (order within each queue = priority) ----
            sv = pp.tile([128, nsv], f32, name="sv", tag="sv")
            nc.sync.dma_start(sv[:], sv_d[:])
            bq_t = [sv[:, j:j + 1] for j in range(3)]
            bk_t = [sv[:, 3 + j:4 + j] for j in range(3)]
            mk_t = [sv[:, 6 + k:7 + k] for k in range(bkt)]
            if has_extra:
                mke_t = sv[:, 6 + bkt:7 + bkt]
                bqe_t = sv[:, 7 + bkt:8 + bkt]
                bke_t = sv[:, 8 + bkt:9 + bkt]

            xbig = pp.tile([128, 6 * 1024], f16, name="xbig", tag="xbig")
            wqbig = pp.tile([128, 6 * 384], f16, name="wqbig", tag="wqbig")
            wkbig = pp.tile([128, 6 * 384], f16, name="wkbig", tag="wkbig")
            wvbig = pp.tile([128, 6 * 768], f16, name="wvbig", tag="wvbig")
            if has_extra:
                xebig = pp.tile([128, 6 * 1024], f16, name="xebig",
                                tag="xebig")
                xekt = pp.tile([128, 6 * 128], f16, name="xekt", tag="xekt")
                wqe = pp.tile([128, 6 * 128], f16, name="wqe", tag="wqe")
                wke = pp.tile([128, 6 * 128], f16, name="wke", tag="wke")
            if has_cvec:
                ones1 = pp.tile([1, 128], f16, name="ones1", tag="ones1")
                wvo6 = pp.tile([1, 768], f16, name="wvo6", tag="wvo6")

            # sync: wq even + x 0,3 + wk even (+wqe); scalar: wq odd +
            # x 1,4 + wk odd (+wke); gpsimd(SWDGE): x 2,5 + xekt + wvp + xe.
            def wq_piece(i):
                return (wqbig[:, i * 384:(i + 1) * 384],
                        wqp_d[:, i * 384:(i + 1) * 384])

            def wk_piece(i):
                return (wkbig[:, i * 384:(i + 1) * 384],
                        wkp_d[:, i * 384:(i + 1) * 384])

            def x_piece(i):
                return (xbig[:, i * 1024:(i + 1) * 1024],
                        xp_d[:, i * 1024:(i + 1) * 1024])

            # Measured queue rates: SW(gpsimd) ~200 B/ns, each HW queue
            # only ~60 B/ns.  Deliveries are scheduled in consumption
            # order against those rates: QT's (wq_i, x_i) pairs ride the
            # fast SW queue (kt 3 on sync, kt 4 on scalar so the HW queues
            # contribute), then wk on the HW queues (KT runs before VW),
            # then wvp + the extra unit's inputs on the SW queue.
            # HW queues (slow for loads, ~70 B/ns) carry wk first (KT
            # must never stall: a >3.4us PE gap re-throttles the clock)
            # then the kt-3/4 QT pair; the fast SW queue carries the rest
            # of QT's pairs, then wvp, then the extra unit's inputs. The
            # QT loop consumes kt in arrival order (0,1,2,5 then 3,4).
            for i in (0, 1, 2):
                nc.sync.dma_start(*wk_piece(i))
                nc.scalar.dma_start(*wk_piece(3 + i))
            for i in (0, 1, 2):
                nc.gpsimd.dma_start(*wq_piece(i))
                nc.gpsimd.dma_start(*x_piece(i))
            nc.gpsimd.dma_start(*wq_piece(5))
            nc.gpsimd.dma_start(*x_piece(5))
            nc.sync.dma_start(*wq_piece(3))
            nc.sync.dma_start(*x_piece(3))
            nc.scalar.dma_start(*wq_piece(4))
            nc.scalar.dma_start(*x_piece(4))
            if has_cvec:
                nc.vector.memset(ones1[:], 1.0)
                nc.scalar.dma_start(wvo6[:], wvo6_d[:])
            for i in range(6):
                nc.gpsimd.dma_start(wvbig[:, i * 768:(i + 1) * 768],
                                    wvp_d[:, i * 768:(i + 1) * 768])
            if has_extra:
                nc.gpsimd.dma_start(xekt[:], xekt_d[:])
                nc.gpsimd.dma_start(xebig[:], xe_d[:])
                nc.sync.dma_start(wqe[:], wqe_d[:])
                nc.scalar.dma_start(wke[:], wke_d[:])

            xt = [xbig[:, i * 1024:(i + 1) * 1024] for i in range(6)]
            wq_t = [wqbig[:, i * 384:(i + 1) * 384] for i in range(6)]
            xkt = [xbig[:, i * 1024:i * 1024 + KMAX] for i in range(6)]
            wk_t = [wkbig[:, i * 384:(i + 1) * 384] for i in range(6)]
            wvo_t = [wvbig[:, i * 768:(i + 1) * 768] for i in range(6)]
            if has_extra:
                xet = [xebig[:, i * 1024:(i + 1) * 1024] for i in range(6)]

            # persistent intermediates
            QT = [pp.tile([128, 1024], f16, name=f"QT{j}", tag=f"QT{j}")
                  for j in range(3)]
            KT = [pp.tile([128, KMAX], f16, name=f"KT{j}", tag=f"KT{j}")
                  for j in range(3)]
            VW = [pp.tile([128, 770], f16, name=f"VW{m}", tag=f"VW{m}")
                  for m in range(bkt)]
            if has_extra:
                QTe = pp.tile([128, 1024], f16, name="QTe", tag="QTe")
                KTe = pp.tile([128, 128], f16, name="KTe", tag="KTe")
                VWe = pp.tile([128, 770], f16, name="VWe", tag="VWe")

            class _QkPool:
                """Adapter: score tiles drawn from psA's qk banks (free
                after QTe) so chunk 0's scores+exps run inside phase A."""
                def __init__(self, pool):
                    self.pool = pool

                def tile(self, shape, dt, name, tag):
                    return self.pool.tile(shape, dt, name=name, tag="qk")

            def emit_scores(ch, sp):
                """Score MMs kt-major (2-head row-group concurrency),
                then exps hh-major so the first PV group unblocks
                after only 4 exps. Fills ch['ut']. sp: the PSUM pool
                to draw score tiles from."""
                nkt = len(ch["kt_sb"])
                qt_sb, qch, masks = ch["qt"], ch["qch"], ch["masks"]
                psS = [[None] * nkt for _ in range(2)]
                for i in range(nkt):
                    ktile, csel, _vw = ch["kt_sb"][i]
                    for hh in range(2):
                        p0 = hh * 64
                        ps = sp.tile([128, 512], f32, name="psS",
                                     tag="psS")
                        nc.tensor.matmul(
                            ps[:], ktile[p0:p0 + 64, csel],
                            qt_sb[p0:p0 + 64, qch])
                        psS[hh][i] = ps
                ut = [[None] * nkt for _ in range(2)]
                # hh-major exps let the first PV group start after
                # only nkt exps; beyond 4 tiles that ordering inverts
                # the psS buffer-recycle dependencies against the
                # in-order queues (deadlock), so fall back to the MM
                # allocation order (kt-major)
                if nkt <= 4:
                    order = [(hh, i) for hh in range(2)
                             for i in range(nkt)]
                else:
                    order = [(hh, i) for i in range(nkt)
                             for hh in range(2)]
                for hh, i in order:
                    u = utp.tile([128, 512], f16, name="ut", tag="ut")
                    nc.scalar.activation(
                        u[:], psS[hh][i][:], AF.Exp, bias=masks[i])
                    ut[hh][i] = u
                ch["ut"] = ut

            base_rings = (nc.sync, nc.gpsimd)
            chunks = []
            for ci, (j, qc) in enumerate(
                    (j, qc) for j in range(3) for qc in range(2)):
                def odst(mp, hh, j=j, qc=qc):
                    return out_d[j, hh, qc, :, ds(mp * 2, 2), :]
                def odstf(hh, j=j, qc=qc):
                    return out_d[j, hh, qc, :, :, :]
                chunks.append(dict(
                    qch=ds(qc * 512, 512),
                    kt_sb=[(KT[j], ts(kt, 128), VW[kt])
                           for kt in range(bkt)],
                    qt=QT[j], masks=mk_t, odst=odst, odstf=odstf,
                    small=False, last=False, rings=base_rings))
                if has_extra and ci < 2:
                    def eodst(mp, hh, qc=ci):
                        return oute_d[hh, qc, :, ds(mp * 2, 2), :]
                    def eodstf(hh, qc=ci):
                        return oute_d[hh, qc, :, :, :]
                    chunks.append(dict(
                        qch=ds(ci * 512, 512),
                        kt_sb=[(KTe, ds(0, 128), VWe)],
                        qt=QTe, masks=[mke_t], odst=eodst, odstf=eodstf,
                        small=True, last=False, rings=base_rings))
            # last chunk: finer DMA granularity over all 3 queues so the
            # post-final-matmul tail is short
            chunks[-1]["rings"] = (nc.sync, nc.gpsimd, nc.scalar)
            chunks[-1]["last"] = True

            # ---- phase A: projections ----
            # PE warm-up: junk matmuls keep the tensor engine busy through
            # the HAM SHORT window (~3.4us) AND until the first x/wq
            # pieces land (~10us): 35 N=512 matmuls span ~8 cold + ~27
            # warm ~= 9.2us, so real work starts at 2.4 GHz with no
            # re-throttling idle gap.  memset on gpsimd (earliest-ready
            # engine) so the first warmup MM issues at ~1.3us.
            wsrc = pp.tile([128, 512], f16, name="wsrc", tag="wsrc")
            nc.gpsimd.memset(wsrc[:], 0.0)
            with tc.tile_pool(name="psA", bufs=6, space="PSUM") as psA:
                # warmup junk shares the "vw" tag's banks -- no separate
                # pool, so no pool-close drain between warmup and QT
                for _ in range(35):
                    psw = psA.tile([64, 512], f32, name="warm", tag="vw",
                                   bufs=2)
                    nc.tensor.matmul(psw[:], wsrc[:, 0:64], wsrc[:])
                # QT is kt-major: all six (j,qc) PSUM groups accumulate in
                # parallel so each arriving x tile is consumed immediately.
                qgroups = [(j, qc) for j in range(3) for qc in range(2)]
                qps = [psA.tile([128, 512], f32, name=f"qtp{j}{qc}", tag="qk")
                       for j, qc in qgroups]
                kt_order = (0, 1, 2, 5, 3, 4)   # input arrival order
                for ki, kt in enumerate(kt_order):
                    for gi, (j, qc) in enumerate(qgroups):
                        nc.tensor.matmul(
                            qps[gi][:], wq_t[kt][:, ts(j, 128)],
                            xt[kt][:, ds(qc * 512, 512)],
                            start=(ki == 0), stop=(ki == 5))
                for gi, (j, qc) in enumerate(qgroups):
                    nc.scalar.activation(
                        QT[j][:, ds(qc * 512, 512)], qps[gi][:], AF.Identity,
                        bias=bq_t[j])
                def emit_vw(dst, src_tiles, msel):
                    # dst[k,f] accumulates src.T @ (Wv@Wo) for one key
                    # tile; cols 768:770 are BOTH ones (the PV chains
                    # split 385+385, each f-half carrying a sigma column)
                    for fo, w in ((0, 385), (385, 383)):
                        fch = ds(fo, w)
                        ps = psA.tile([128, 385], f32, name="vw", tag="vw",
                                      bufs=2)
                        for kt in range(6):
                            nc.tensor.matmul(
                                ps[:, 0:w], src_tiles[kt][:, msel],
                                wvo_t[kt][:, fch],
                                start=(kt == 0),
                                stop=(kt == 5 and not has_cvec))
                        if has_cvec:
                            nc.tensor.matmul(
                                ps[:, 0:w], ones1[:], wvo6[:, fch],
                                start=False, stop=True)
                        nc.vector.tensor_copy(dst[:, fch], ps[:, 0:w])
                    nc.vector.memset(dst[:, 768:770], 1.0)

                def emit_kt(j):
                    for o, w in kchunks:
                        kch = ds(o, w)
                        ps2 = psA.tile([128, 512], f32, name="ktp", tag="qk")
                        for kt in range(6):
                            nc.tensor.matmul(
                                ps2[:, 0:w], wk_t[kt][:, ts(j, 128)],
                                xkt[kt][:, kch],
                                start=(kt == 0), stop=(kt == 5))
                        nc.scalar.activation(
                            KT[j][:, kch], ps2[:, 0:w], AF.Identity,
                            bias=bk_t[j])

                class _QkPool:
                    """Adapter: chunk 0's score tiles draw from psA's qk
                    ring so scores+exps run inside phase A (the exps
                    overlap VW/extra projections on the scalar engine and
                    PV can start the moment phase B's pools open)."""
                    def tile(self, shape, dt, name, tag):
                        return psA.tile(shape, dt, name=name, tag="qk")

                emit_kt(0)
                emit_scores(chunks[0], _QkPool())
                emit_kt(1)
                emit_kt(2)
                # KT first (wk arrives early on the HW queues), then VW
                # (wvp lands on the SW queue by ~14us).
                for m in range(bkt):
                    emit_vw(VW[m], xkt, ts(m, 128))
                if has_extra:
                    emit_vw(VWe, [xekt[:, ts(i, 128)] for i in range(6)],
                            ds(0, 128))
                if has_extra:
                    # extra pair's K^T over its single key tile
                    pse = psA.tile([128, 128], f32, name="kte", tag="qk")
                    for kt in range(6):
                        nc.tensor.matmul(
                            pse[:], wke[:, ts(kt, 128)],
                            xekt[:, ts(kt, 128)],
                            start=(kt == 0), stop=(kt == 5))
                    nc.scalar.activation(KTe[:], pse[:], AF.Identity,
                                         bias=bke_t)
                    # extra pair's Q^T over all 1024 extra-batch tokens;
                    # last in phase A -- xe is the latest-arriving input
                    for qc in range(2):
                        psq = psA.tile([128, 512], f32, name="qte", tag="qk")
                        for kt in range(6):
                            nc.tensor.matmul(
                                psq[:], wqe[:, ts(kt, 128)],
                                xet[kt][:, ds(qc * 512, 512)],
                                start=(kt == 0), stop=(kt == 5))
                        nc.scalar.activation(
                            QTe[:, ds(qc * 512, 512)], psq[:], AF.Identity,
                            bias=bqe_t)

            # ---- phase B: attention ----
            ring_i = [0]

            def out_dma(dst, src, rings):
                rings[ring_i[0] % len(rings)].dma_start(dst, src)
                ring_i[0] += 1

            drain_i = [0]
            zb = pp.tile([128, 1], f32, name="zb", tag="zb")
            nc.gpsimd.memset(zb[:], 0.0)

            def drain(dst, src):
                # PSUM->SBUF drains 2:1 vector:scalar -- keeps the vector
                # queue short so psO banks recycle without stalling PV
                if drain_i[0] % 3 < 2:
                    nc.vector.tensor_copy(dst, src)
                else:
                    nc.scalar.activation(dst, src, AF.Identity, bias=zb[:])
                drain_i[0] += 1

            with (
                tc.tile_pool(name="psS", bufs=4, space="PSUM") as psSp,
                tc.tile_pool(name="psO", bufs=2, space="PSUM") as psOp,
            ):

                def emit_pv(ch, nxt):
                    """PV groups hh-major; each group accumulates into one
                    2-bank PSUM tile (pa bank 0, pb bank-aligned at col
                    512), drained by ONE strided cast; output DMAs go ONE
                    per (hh) (per mq-pair on the last chunk for a short
                    tail). The NEXT chunk's scores+exps are hoisted in
                    between the hh halves so its exps complete during this
                    chunk's PV (a chunk-boundary exp wait re-throttles the
                    PE clock)."""
                    nkt = len(ch["kt_sb"])
                    ut, rings = ch["ut"], ch["rings"]
                    for hh in range(2):
                        oh = op_.tile([128, 4, 770], f16, name="ob",
                                      tag="ob")
                        for mq in range(4):
                            po = psOp.tile([128, 2, 512], f32, name="psO",
                                           tag="psO")
                            pa = po[:, 0, 0:385]
                            pb = po[:, 1, 0:385]
                            for i in range(nkt):
                                nc.tensor.matmul(
                                    pb, ut[hh][i][:, ts(mq, 128)],
                                    ch["kt_sb"][i][2][:, 385:770],
                                    start=(i == 0), stop=(i == nkt - 1))
                            for i in range(nkt):
                                nc.tensor.matmul(
                                    pa, ut[hh][i][:, ts(mq, 128)],
                                    ch["kt_sb"][i][2][:, 0:385],
                                    start=(i == 0), stop=(i == nkt - 1))
                            drain(oh[:, mq, :], po[:, :, 0:385])
                            if ch["last"] and mq % 2 == 1:
                                out_dma(ch["odst"](mq // 2, hh),
                                        oh[:, ds(mq - 1, 2), :], rings)
                        if not ch["last"]:
                            out_dma(ch["odstf"](hh), oh[:], rings)
                        if hh == 0 and nxt is not None:
                            emit_scores(nxt, psSp)

                for ci, ch in enumerate(chunks):
                    emit_pv(ch, chunks[ci + 1] if ci + 1 < len(chunks)
                            else None)
    nc.compile()
    return nc


def get_program(bkt, has_extra, has_cvec):
    key = (bkt, has_extra, has_cvec)
    if key not in _PROGRAM_CACHE:
        _PROGRAM_CACHE[key] = _build_program(*key)
    return _PROGRAM_CACHE[key]


def _pair_cols(w, pair, scale=1.0):
    """Wq/Wk columns for one head pair -> packed [128, 6*128] f16."""
    cs = w[:, pair * 128:(pair + 1) * 128] * scale
    return _pack6(cs.astype(np.float16))


def prep(x, mask, Wq, bq, Wk, bk, Wv, bv, Wo, bo):
    """Host-side sharding/compaction. Returns (bkt, has_extra, has_cvec,
    in_maps, perms, extras) where extras[c] = (batch, pair) or None."""
    f16 = np.float16
    x = np.asarray(x, np.float32)
    mask = np.asarray(mask)
    Wq = np.asarray(Wq, np.float32)
    Wk = np.asarray(Wk, np.float32)
    Wv = np.asarray(Wv, np.float32)
    Wo = np.asarray(Wo, np.float32)
    bq = np.asarray(bq, np.float32)
    bk = np.asarray(bk, np.float32)
    bv = np.asarray(bv, np.float32)
    bo = np.asarray(bo, np.float32)

    mrow = [mask[b, 0, 0] != 0 for b in range(B)]
    perms = [np.argsort(~mrow[b], kind="stable") for b in range(B)]
    nkeep = [int(mrow[b].sum()) for b in range(B)]
    tb = [min(8, max(1, math.ceil(n / 128))) for n in nkeep]
    tmax = max(tb)
    # overflow (batch, pair) units if base = tmax-1; one extra slot per
    # core when the total fits in 8, else fall back to the uniform max.
    ov = [(b, p) for b in range(B) if tb[b] == tmax for p in range(6)]
    if tmax > 1 and 0 < len(ov) <= NCORES and min(tb) < tmax:
        bkt, has_extra = tmax - 1, True
    else:
        bkt, has_extra = tmax, False
        ov = []
    KMAX = 128 * bkt

    cvec = bv @ Wo + bo
    has_cvec = bool(np.any(cvec))

    wq_p, wk_p, bq_p, bk_p = [], [], [], []
    for g in range(2):
        cs = slice(g * GW, (g + 1) * GW)
        wq_p.append(_pack6((Wq[:, cs] * 0.125).astype(f16)))
        wk_p.append(_pack6(Wk[:, cs].astype(f16)))
        bq_p.append((bq[cs] * 0.125).reshape(3, 128).T)   # [128,3]
        bk_p.append(bk[cs].reshape(3, 128).T)
    wvp = _pack6((Wv @ Wo).astype(f16))
    wvo6 = cvec.astype(f16).reshape(1, 768)

    xp_b, mk_b = [], []
    for b in range(B):
        xp_b.append(_pack6(x[b][perms[b]].T.astype(f16)))
        mk = np.full(KMAX, -1e9, np.float32)
        mk[:min(nkeep[b], KMAX)] = 0.0
        mk_b.append(mk.reshape(bkt, 128).T)

    # extra-slot assignment: overflow units go to non-owner cores first
    extras = [None] * NCORES
    if has_extra:
        order = sorted(range(NCORES), key=lambda c: c // 2 in
                       {b for b, _ in ov})
        for slot, unit in zip(order, ov):
            extras[slot] = unit

    def xe_tile(b):
        """Packed [128, 6*128] of the extra key-tile tokens of batch b."""
        lo = bkt * 128
        xs = np.zeros((128, 768), np.float32)
        hi = min(1024, lo + 128)
        xs[:hi - lo] = x[b][perms[b][lo:hi]]
        return _pack6(np.ascontiguousarray(xs.T).astype(f16))

    nsv = 6 + bkt + (3 if has_extra else 0)
    in_maps = []
    for c in range(NCORES):
        b, g = c // 2, c % 2
        sv = np.zeros((128, nsv), np.float32)
        sv[:, 0:3] = bq_p[g]
        sv[:, 3:6] = bk_p[g]
        sv[:, 6:6 + bkt] = mk_b[b]
        im = {"xp": xp_b[b], "wqp": wq_p[g], "wkp": wk_p[g], "wvp": wvp}
        if has_cvec:
            im["wvo6"] = wvo6
        if has_extra:
            if extras[c] is not None:
                eb, ep = extras[c]
                mke = np.full(128, -1e9, np.float32)
                ner = nkeep[eb] - bkt * 128
                mke[:max(0, min(128, ner))] = 0.0
                sv[:, 6 + bkt] = mke
                sv[:, 7 + bkt] = bq[ep * 128:(ep + 1) * 128] * 0.125
                sv[:, 8 + bkt] = bk[ep * 128:(ep + 1) * 128]
                im["xe"] = xp_b[eb]
                im["xekt"] = xe_tile(eb)
                im["wqe"] = _pair_cols(Wq, ep, 0.125)
                im["wke"] = _pair_cols(Wk, ep)
            else:
                sv[:, 6 + bkt] = -1e9   # dummy: fully masked
                im["xe"] = xp_b[b]
                im["xekt"] = xe_tile(b)
                im["wqe"] = _pair_cols(Wq, 0, 0.125)
                im["wke"] = _pair_cols(Wk, 0)
        im["sv"] = sv
        in_maps.append(im)
    return bkt, has_extra, has_cvec, in_maps, perms, extras


def gather_output(results, perms, extras):
    num = np.zeros((B, NH, S, 768), np.float32)
    sig = np.zeros((B, NH, S, 1), np.float32)
    def unshuffle(o):
        # [.., hh, qc, p, mq, f] -> [.., hh, q, f] with q = qc*512+mq*128+p
        o = o.transpose(0, 1, 2, 4, 3, 5)        # [j, hh, qc, mq, p, f]
        return o.reshape(o.shape[0], 2, 1024, 770)

    for c in range(NCORES):
        b, g = c // 2, c % 2
        o = unshuffle(np.asarray(results[c]["out"], np.float32))
        for j in range(3):
            for hh in range(2):
                h = g * 6 + j * 2 + hh
                num[b, h] += o[j, hh, :, :768]
                sig[b, h, :, 0] += o[j, hh, :, 768]
        if extras[c] is not None:
            eb, ep = extras[c]
            oe = unshuffle(np.asarray(
                results[c]["oute"], np.float32)[None])[0]
            for hh in range(2):
                h = ep * 2 + hh
                num[eb, h] += oe[hh, :, :768]
                sig[eb, h, :, 0] += oe[hh, :, 768]
    res = num / sig                                    # [B,NH,S,H]
    out = np.empty((B, S * NH, H), np.float32)
    ov = out.reshape(B, S, NH, H)
    for b in range(B):
        ov[b, perms[b]] = res[b].transpose(1, 0, 2)
    return out


def kernel(**inputs):
    from concourse.bass_utils import run_bass_kernel_spmd

    bkt, has_extra, has_cvec, in_maps, perms, extras = prep(**inputs)
    nc = get_program(bkt, has_extra, has_cvec)
    res = run_bass_kernel_spmd(nc, in_maps, core_ids=list(range(NCORES)))
    return gather_output(res.results, perms, extras)


if __name__ == "__main__":
    rng = np.random.default_rng(0)
    demo = {
        "x": rng.standard_normal((B, S, H), dtype=np.float32),
        "mask": rng.integers(0, 2, (B, 1, 1, S)).astype(np.int32),
        "Wq": rng.standard_normal((H, H), dtype=np.float32) / np.sqrt(H),
        "bq": np.zeros(H, np.float32),
        "Wk": rng.standard_normal((H, H), dtype=np.float32) / np.sqrt(H),
        "bk": np.zeros(H, np.float32),
        "Wv": rng.standard_normal((H, H), dtype=np.float32) / np.sqrt(H),
        "bv": np.zeros(H, np.float32),
        "Wo": rng.standard_normal((H, H), dtype=np.float32) / np.sqrt(H),
        "bo": np.zeros(H, np.float32),
    }
    out = kernel(**demo)
    print("kernel ran, output shape", out.shape)



# revision 15
# speedup vs baseline: 1.0643x; 1.0643x over previous
"""Trainium2 Bass kernel for the quirky MultiHeadAttention module.

Reference computation (B=4, S=1024, H=768, NH=12, HS=64):
    Q = (x@Wq+bq)  split into heads     [B,12,S,64]
    K = (x@Wk+bk)  split into heads     [B,12,S,64]
    V = x@Wv+bv    NOT split            [B,S,768]
    A = softmax(QK^T/8 + mask)          [B,12,S,S]
    out = (A @ V) reshaped [B, S*12, H] @ Wo + bo    -> [4, 12288, 768]

Algebraic restructuring:
  * (A @ V) @ Wo = A @ (x @ (Wv@Wo)) (+ cvec = bv@Wo + bo, realized by a
    constant row added to VW -- softmax rows sum to one).
  * Masked keys produce exp(-1e9+s) == 0 exactly, so they are dropped on
    the host and the key axis is compacted per batch.
  * The softmax denominator comes from a ones-column appended to VW.
    The device emits UNNORMALIZED numerators + the sigma column in f16;
    the host performs the division (allows partial-sum sharding).

Sharding: 8 cores = 4 batches x 2 head-groups (6 heads each), pure SPMD.
Key tiles are 128 wide. The base program uses B_kt tiles per core; when
exactly one "tile level" of overflow exists (e.g. per-batch tile counts
[4,4,4,5]), the overflow (batch, head-pair, tile) units are farmed out
one-per-core as an "extra unit" (cores of the overflowing batch run a
fully-masked dummy), and the host sums the unnormalized partials. This
removes the padding waste of a uniform max-tile program.

Layouts (all transposed so no on-device transposes are needed):
    QT/KT: [feat, tok] with head pairs packed 64+64 in partitions; the
        64-row score matmuls run 2-head-concurrent via PE row groups.
    S^T = KT.T @ QT -> [k, q]  (k on partitions => mask is a per-partition
        bias folded into the Exp activation)
    U = exp(S^T) [k, q] f16 -> exactly the layout the PV matmul needs
    out = U.T @ [VW | 1] -> [q, 769] with col 768 = sigma, stored f16.
Matmul operands are f16 (same PE speed as bf16, ~4x less rounding error
than bf16); accumulation is f32 in PSUM. Inputs are packed host-side into
partition-major [128, N] blobs and streamed over all three DMA queues
(sync/scalar HWDGE + gpsimd SWDGE) in consumption order; a burst of tiny
matmuls pre-warms the PE clock gate (HAM) to 2.4 GHz before real work.
"""

import math

import numpy as np

B, S, H, NH, HS = 4, 1024, 768, 12, 64
GW = 384          # head-group width = 6 heads * 64
NCORES = 8

_PROGRAM_CACHE = {}


def _pack6(a):
    """[768, N] -> partition-major [128, 6*N] (tile i at cols i*N:(i+1)*N)."""
    n = a.shape[1]
    return np.ascontiguousarray(
        a.reshape(6, 128, n).transpose(1, 0, 2).reshape(128, 6 * n))



def _build_program(bkt, has_extra, has_cvec):
    """bkt: number of base 128-wide key tiles per core (1..8).
    has_extra: include one (pair, tile) overflow unit per core.
    has_cvec: include the rank-1 (bv@Wo + bo) constant row in VW."""
    import concourse.mybir as mybir
    import concourse.tile as tile
    from concourse import bacc
    from concourse.bass import ds, ts

    f32 = mybir.dt.float32
    f16 = mybir.dt.float16
    AF = mybir.ActivationFunctionType

    KMAX = 128 * bkt
    if KMAX <= 512:
        kchunks = [(0, KMAX)]
    else:
        w1 = 128 * ((bkt + 1) // 2)
        kchunks = [(0, w1), (w1, KMAX - w1)]
    # sv columns: bq(3) bk(3) mk(bkt) [mke bqe bke]
    nsv = 6 + bkt + (3 if has_extra else 0)

    nc = bacc.Bacc(None, target_bir_lowering=False, debug=False)

    xp_d = nc.dram_tensor("xp", (128, 6 * 1024), f16, kind="ExternalInput")
    wqp_d = nc.dram_tensor("wqp", (128, 6 * 384), f16, kind="ExternalInput")
    wkp_d = nc.dram_tensor("wkp", (128, 6 * 384), f16, kind="ExternalInput")
    wvp_d = nc.dram_tensor("wvp", (128, 6 * 768), f16, kind="ExternalInput")
    sv_d = nc.dram_tensor("sv", (128, nsv), f32, kind="ExternalInput")
    if has_cvec:
        wvo6_d = nc.dram_tensor("wvo6", (1, 768), f16, kind="ExternalInput")
    if has_extra:
        xe_d = nc.dram_tensor("xe", (128, 6 * 1024), f16, kind="ExternalInput")
        xekt_d = nc.dram_tensor("xekt", (128, 6 * 128), f16,
                                kind="ExternalInput")
        wqe_d = nc.dram_tensor("wqe", (128, 6 * 128), f16,
                               kind="ExternalInput")
        wke_d = nc.dram_tensor("wke", (128, 6 * 128), f16,
                               kind="ExternalInput")
    # layout [j, hh, qc, p, mq, f]: DMA src iterates (partition, mq-block,
    # f), so those must be the three minor dims of the destination
    out_d = nc.dram_tensor("out", (3, 2, 2, 128, 4, 770), f16,
                           kind="ExternalOutput")
    if has_extra:
        oute_d = nc.dram_tensor("oute", (2, 2, 128, 4, 770), f16,
                                kind="ExternalOutput")

    with tile.TileContext(nc) as tc:
        with (
            tc.tile_pool(name="persist", bufs=1) as pp,
            tc.tile_pool(name="ut", bufs=4 * max(bkt, 2)) as utp,
            tc.tile_pool(name="osb", bufs=4) as op_,
        ):
            # ---- stream inputs (order within each queue = priority) ----
            sv = pp.tile([128, nsv], f32, name="sv", tag="sv")
            nc.sync.dma_start(sv[:], sv_d[:])
            bq_t = [sv[:, j:j + 1] for j in range(3)]
            bk_t = [sv[:, 3 + j:4 + j] for j in range(3)]
            mk_t = [sv[:, 6 + k:7 + k] for k in range(bkt)]
            if has_extra:
                mke_t = sv[:, 6 + bkt:7 + bkt]
                bqe_t = sv[:, 7 + bkt:8 + bkt]
                bke_t = sv[:, 8 + bkt:9 + bkt]

            xbig = pp.tile([128, 6 * 1024], f16, name="xbig", tag="xbig")
            wqbig = pp.tile([128, 6 * 384], f16, name="wqbig", tag="wqbig")
            wkbig = pp.tile([128, 6 * 384], f16, name="wkbig", tag="wkbig")
            wvbig = pp.tile([128, 6 * 768], f16, name="wvbig", tag="wvbig")
            if has_extra:
                xebig = pp.tile([128, 6 * 1024], f16, name="xebig",
                                tag="xebig")
                xekt = pp.tile([128, 6 * 128], f16, name="xekt", tag="xekt")
                wqe = pp.tile([128, 6 * 128], f16, name="wqe", tag="wqe")
                wke = pp.tile([128, 6 * 128], f16, name="wke", tag="wke")
            if has_cvec:
                ones1 = pp.tile([1, 128], f16, name="ones1", tag="ones1")
                wvo6 = pp.tile([1, 768], f16, name="wvo6", tag="wvo6")

            # sync: wq even + x 0,3 + wk even (+wqe); scalar: wq odd +
            # x 1,4 + wk odd (+wke); gpsimd(SWDGE): x 2,5 + xekt + wvp + xe.
            def wq_piece(i):
                return (wqbig[:, i * 384:(i + 1) * 384],
                        wqp_d[:, i * 384:(i + 1) * 384])

            def wk_piece(i):
                return (wkbig[:, i * 384:(i + 1) * 384],
                        wkp_d[:, i * 384:(i + 1) * 384])

            def x_piece(i):
                return (xbig[:, i * 1024:(i + 1) * 1024],
                        xp_d[:, i * 1024:(i + 1) * 1024])

            # Measured queue rates: SW(gpsimd) ~200 B/ns, each HW queue
            # only ~60 B/ns.  Deliveries are scheduled in consumption
            # order against those rates: QT's (wq_i, x_i) pairs ride the
            # fast SW queue (kt 3 on sync, kt 4 on scalar so the HW queues
            # contribute), then wk on the HW queues (KT runs before VW),
            # then wvp + the extra unit's inputs on the SW queue.
            # HW queues (slow for loads, ~70 B/ns) carry wk first (KT
            # must never stall: a >3.4us PE gap re-throttles the clock)
            # then the kt-3/4 QT pair; the fast SW queue carries the rest
            # of QT's pairs, then wvp, then the extra unit's inputs. The
            # QT loop consumes kt in arrival order (0,1,2,5 then 3,4).
            for i in (0, 1, 2):
                nc.sync.dma_start(*wk_piece(i))
                nc.scalar.dma_start(*wk_piece(3 + i))
            for i in (0, 1, 2):
                nc.gpsimd.dma_start(*wq_piece(i))
                nc.gpsimd.dma_start(*x_piece(i))
            nc.gpsimd.dma_start(*wq_piece(5))
            nc.gpsimd.dma_start(*x_piece(5))
            nc.sync.dma_start(*wq_piece(3))
            nc.sync.dma_start(*x_piece(3))
            nc.scalar.dma_start(*wq_piece(4))
            nc.scalar.dma_start(*x_piece(4))
            if has_cvec:
                nc.vector.memset(ones1[:], 1.0)
                nc.scalar.dma_start(wvo6[:], wvo6_d[:])
            for i in range(6):
                nc.gpsimd.dma_start(wvbig[:, i * 768:(i + 1) * 768],
                                    wvp_d[:, i * 768:(i + 1) * 768])
            if has_extra:
                nc.gpsimd.dma_start(xekt[:], xekt_d[:])
                nc.gpsimd.dma_start(xebig[:], xe_d[:])
                nc.sync.dma_start(wqe[:], wqe_d[:])
                nc.scalar.dma_start(wke[:], wke_d[:])

            xt = [xbig[:, i * 1024:(i + 1) * 1024] for i in range(6)]
            wq_t = [wqbig[:, i * 384:(i + 1) * 384] for i in range(6)]
            xkt = [xbig[:, i * 1024:i * 1024 + KMAX] for i in range(6)]
            wk_t = [wkbig[:, i * 384:(i + 1) * 384] for i in range(6)]
            wvo_t = [wvbig[:, i * 768:(i + 1) * 768] for i in range(6)]
            if has_extra:
                xet = [xebig[:, i * 1024:(i + 1) * 1024] for i in range(6)]

            # persistent intermediates
            QT = [pp.tile([128, 1024], f16, name=f"QT{j}", tag=f"QT{j}")
                  for j in range(3)]
            KT = [pp.tile([128, KMAX], f16, name=f"KT{j}", tag=f"KT{j}")
                  for j in range(3)]
            VW = [pp.tile([128, 770], f16, name=f"VW{m}", tag=f"VW{m}")
                  for m in range(bkt)]
            if has_extra:
                QTe = pp.tile([128, 1024], f16, name="QTe", tag="QTe")
                KTe = pp.tile([128, 128], f16, name="KTe", tag="KTe")
                VWe = pp.tile([128, 770], f16, name="VWe", tag="VWe")

            class _QkPool:
                """Adapter: score tiles drawn from psA's qk banks (free
                after QTe) so chunk 0's scores+exps run inside phase A."""
                def __init__(self, pool):
                    self.pool = pool

                def tile(self, shape, dt, name, tag):
                    return self.pool.tile(shape, dt, name=name, tag="qk")

            def emit_scores(ch, sp):
                """Score MMs kt-major (2-head row-group concurrency),
                then exps hh-major so the first PV group unblocks
                after only 4 exps. Fills ch['ut']. sp: the PSUM pool
                to draw score tiles from."""
                nkt = len(ch["kt_sb"])
                qt_sb, qch, masks = ch["qt"], ch["qch"], ch["masks"]
                psS = [[None] * nkt for _ in range(2)]
                for i in range(nkt):
                    ktile, csel, _vw = ch["kt_sb"][i]
                    for hh in range(2):
                        p0 = hh * 64
                        ps = sp.tile([128, 512], f32, name="psS",
                                     tag="psS")
                        nc.tensor.matmul(
                            ps[:], ktile[p0:p0 + 64, csel],
                            qt_sb[p0:p0 + 64, qch])
                        psS[hh][i] = ps
                ut = [[None] * nkt for _ in range(2)]
                # hh-major exps let the first PV group start after
                # only nkt exps; beyond 4 tiles that ordering inverts
                # the psS buffer-recycle dependencies against the
                # in-order queues (deadlock), so fall back to the MM
                # allocation order (kt-major)
                if nkt <= 4:
                    order = [(hh, i) for hh in range(2)
                             for i in range(nkt)]
                else:
                    order = [(hh, i) for i in range(nkt)
                             for hh in range(2)]
                for hh, i in order:
                    u = utp.tile([128, 512], f16, name="ut", tag="ut")
                    nc.scalar.activation(
                        u[:], psS[hh][i][:], AF.Exp, bias=masks[i])
                    ut[hh][i] = u
                ch["ut"] = ut

            base_rings = (nc.sync, nc.gpsimd)
            chunks = []
            for ci, (j, qc) in enumerate(
                    (j, qc) for j in range(3) for qc in range(2)):
                def odst(mp, hh, j=j, qc=qc):
                    return out_d[j, hh, qc, :, ds(mp * 2, 2), :]
                def odstf(hh, j=j, qc=qc):
                    return out_d[j, hh, qc, :, :, :]
                chunks.append(dict(
                    qch=ds(qc * 512, 512),
                    kt_sb=[(KT[j], ts(kt, 128), VW[kt])
                           for kt in range(bkt)],
                    qt=QT[j], masks=mk_t, odst=odst, odstf=odstf,
                    small=False, last=False, rings=base_rings))
                if has_extra and ci < 2:
                    def eodst(mp, hh, qc=ci):
                        return oute_d[hh, qc, :, ds(mp * 2, 2), :]
                    def eodstf(hh, qc=ci):
                        return oute_d[hh, qc, :, :, :]
                    chunks.append(dict(
                        qch=ds(ci * 512, 512),
                        kt_sb=[(KTe, ds(0, 128), VWe)],
                        qt=QTe, masks=[mke_t], odst=eodst, odstf=eodstf,
                        small=True, last=False, rings=base_rings))
            # last chunk: finer DMA granularity over all 3 queues so the
            # post-final-matmul tail is short
            chunks[-1]["rings"] = (nc.sync, nc.gpsimd, nc.scalar)
            chunks[-1]["last"] = True

            # ---- phase A: projections ----
            # PE warm-up: small junk matmuls keep the tensor engine busy
            # through the HAM SHORT window (~3.4us) so real work runs at
            # 2.4 GHz from the start (count tuned to end ~when the first
            # x/wq pieces land).
            wsrc = pp.tile([128, 384], f16, name="wsrc", tag="wsrc")
            nc.vector.memset(wsrc[:], 0.0)
            with tc.tile_pool(name="psA", bufs=6, space="PSUM") as psA:
                # warmup junk shares the "vw" tag's banks -- no separate
                # pool, so no pool-close drain between warmup and QT
                for _ in range(30):
                    psw = psA.tile([64, 128], f32, name="warm", tag="vw",
                                   bufs=2)
                    nc.tensor.matmul(psw[:], wsrc[:, 0:64], wsrc[:, 0:128])
                # QT is kt-major: all six (j,qc) PSUM groups accumulate in
                # parallel so each arriving x tile is consumed immediately.
                qgroups = [(j, qc) for j in range(3) for qc in range(2)]
                qps = [psA.tile([128, 512], f32, name=f"qtp{j}{qc}", tag="qk")
                       for j, qc in qgroups]
                kt_order = (0, 1, 2, 5, 3, 4)   # input arrival order
                for ki, kt in enumerate(kt_order):
                    for gi, (j, qc) in enumerate(qgroups):
                        nc.tensor.matmul(
                            qps[gi][:], wq_t[kt][:, ts(j, 128)],
                            xt[kt][:, ds(qc * 512, 512)],
                            start=(ki == 0), stop=(ki == 5))
                for gi, (j, qc) in enumerate(qgroups):
                    nc.scalar.activation(
                        QT[j][:, ds(qc * 512, 512)], qps[gi][:], AF.Identity,
                        bias=bq_t[j])
                def emit_vw(dst, src_tiles, msel):
                    # dst[k,f] accumulates src.T @ (Wv@Wo) for one key
                    # tile; cols 768:770 are BOTH ones (the PV chains
                    # split 385+385, each f-half carrying a sigma column)
                    for fo, w in ((0, 385), (385, 383)):
                        fch = ds(fo, w)
                        ps = psA.tile([128, 385], f32, name="vw", tag="vw",
                                      bufs=2)
                        for kt in range(6):
                            nc.tensor.matmul(
                                ps[:, 0:w], src_tiles[kt][:, msel],
                                wvo_t[kt][:, fch],
                                start=(kt == 0),
                                stop=(kt == 5 and not has_cvec))
                        if has_cvec:
                            nc.tensor.matmul(
                                ps[:, 0:w], ones1[:], wvo6[:, fch],
                                start=False, stop=True)
                        nc.vector.tensor_copy(dst[:, fch], ps[:, 0:w])
                    nc.vector.memset(dst[:, 768:770], 1.0)

                def emit_kt(j):
                    for o, w in kchunks:
                        kch = ds(o, w)
                        ps2 = psA.tile([128, 512], f32, name="ktp", tag="qk")
                        for kt in range(6):
                            nc.tensor.matmul(
                                ps2[:, 0:w], wk_t[kt][:, ts(j, 128)],
                                xkt[kt][:, kch],
                                start=(kt == 0), stop=(kt == 5))
                        nc.scalar.activation(
                            KT[j][:, kch], ps2[:, 0:w], AF.Identity,
                            bias=bk_t[j])

                class _QkPool:
                    """Adapter: chunk 0's score tiles draw from psA's qk
                    ring so scores+exps run inside phase A (the exps
                    overlap VW/extra projections on the scalar engine and
                    PV can start the moment phase B's pools open)."""
                    def tile(self, shape, dt, name, tag):
                        return psA.tile(shape, dt, name=name, tag="qk")

                emit_kt(0)
                emit_scores(chunks[0], _QkPool())
                emit_kt(1)
                emit_kt(2)
                # KT first (wk arrives early on the HW queues), then VW
                # (wvp lands on the SW queue by ~14us).
                for m in range(bkt):
                    emit_vw(VW[m], xkt, ts(m, 128))
                if has_extra:
                    emit_vw(VWe, [xekt[:, ts(i, 128)] for i in range(6)],
                            ds(0, 128))
                if has_extra:
                    # extra pair's K^T over its single key tile
                    pse = psA.tile([128, 128], f32, name="kte", tag="qk")
                    for kt in range(6):
                        nc.tensor.matmul(
                            pse[:], wke[:, ts(kt, 128)],
                            xekt[:, ts(kt, 128)],
                            start=(kt == 0), stop=(kt == 5))
                    nc.scalar.activation(KTe[:], pse[:], AF.Identity,
                                         bias=bke_t)
                    # extra pair's Q^T over all 1024 extra-batch tokens;
                    # last in phase A -- xe is the latest-arriving input
                    for qc in range(2):
                        psq = psA.tile([128, 512], f32, name="qte", tag="qk")
                        for kt in range(6):
                            nc.tensor.matmul(
                                psq[:], wqe[:, ts(kt, 128)],
                                xet[kt][:, ds(qc * 512, 512)],
                                start=(kt == 0), stop=(kt == 5))
                        nc.scalar.activation(
                            QTe[:, ds(qc * 512, 512)], psq[:], AF.Identity,
                            bias=bqe_t)

            # ---- phase B: attention ----
            ring_i = [0]

            def out_dma(dst, src, rings):
                rings[ring_i[0] % len(rings)].dma_start(dst, src)
                ring_i[0] += 1

            drain_i = [0]
            zb = pp.tile([128, 1], f32, name="zb", tag="zb")
            nc.gpsimd.memset(zb[:], 0.0)

            def drain(dst, src):
                # PSUM->SBUF drains 2:1 vector:scalar -- keeps the vector
                # queue short so psO banks recycle without stalling PV
                if drain_i[0] % 3 < 2:
                    nc.vector.tensor_copy(dst, src)
                else:
                    nc.scalar.activation(dst, src, AF.Identity, bias=zb[:])
                drain_i[0] += 1

            with (
                tc.tile_pool(name="psS", bufs=4, space="PSUM") as psSp,
                tc.tile_pool(name="psO", bufs=2, space="PSUM") as psOp,
            ):

                def emit_pv(ch, nxt):
                    """PV groups hh-major; each group accumulates into one
                    2-bank PSUM tile (pa bank 0, pb bank-aligned at col
                    512), drained by ONE strided cast; output DMAs go ONE
                    per (hh) (per mq-pair on the last chunk for a short
                    tail). The NEXT chunk's scores+exps are hoisted in
                    between the hh halves so its exps complete during this
                    chunk's PV (a chunk-boundary exp wait re-throttles the
                    PE clock)."""
                    nkt = len(ch["kt_sb"])
                    ut, rings = ch["ut"], ch["rings"]
                    for hh in range(2):
                        oh = op_.tile([128, 4, 770], f16, name="ob",
                                      tag="ob")
                        for mq in range(4):
                            po = psOp.tile([128, 2, 512], f32, name="psO",
                                           tag="psO")
                            pa = po[:, 0, 0:385]
                            pb = po[:, 1, 0:385]
                            for i in range(nkt):
                                nc.tensor.matmul(
                                    pb, ut[hh][i][:, ts(mq, 128)],
                                    ch["kt_sb"][i][2][:, 385:770],
                                    start=(i == 0), stop=(i == nkt - 1))
                            for i in range(nkt):
                                nc.tensor.matmul(
                                    pa, ut[hh][i][:, ts(mq, 128)],
                                    ch["kt_sb"][i][2][:, 0:385],
                                    start=(i == 0), stop=(i == nkt - 1))
                            if ch["last"]:
                                # strict 1:1 vector/scalar for a 2-lane
                                # parallel tail
                                (nc.vector.tensor_copy(oh[:, mq, :],
                                                       po[:, :, 0:385])
                                 if mq % 2 == 0 else
                                 nc.scalar.activation(oh[:, mq, :],
                                                      po[:, :, 0:385],
                                                      AF.Identity,
                                                      bias=zb[:]))
                            else:
                                drain(oh[:, mq, :], po[:, :, 0:385])
                            if ch["last"] and mq % 2 == 1:
                                out_dma(ch["odst"](mq // 2, hh),
                                        oh[:, ds(mq - 1, 2), :], rings)
                        if not ch["last"]:
                            out_dma(ch["odstf"](hh), oh[:], rings)
                        if hh == 0 and nxt is not None:
                            emit_scores(nxt, psSp)

                for ci, ch in enumerate(chunks):
                    emit_pv(ch, chunks[ci + 1] if ci + 1 < len(chunks)
                            else None)
    nc.compile()
    return nc


def get_program(bkt, has_extra, has_cvec):
    key = (bkt, has_extra, has_cvec)
    if key not in _PROGRAM_CACHE:
        _PROGRAM_CACHE[key] = _build_program(*key)
    return _PROGRAM_CACHE[key]


def _pair_cols(w, pair, scale=1.0):
    """Wq/Wk columns for one head pair -> packed [128, 6*128] f16."""
    cs = w[:, pair * 128:(pair + 1) * 128] * scale
    return _pack6(cs.astype(np.float16))


def prep(x, mask, Wq, bq, Wk, bk, Wv, bv, Wo, bo):
    """Host-side sharding/compaction. Returns (bkt, has_extra, has_cvec,
    in_maps, perms, extras) where extras[c] = (batch, pair) or None."""
    f16 = np.float16
    x = np.asarray(x, np.float32)
    mask = np.asarray(mask)
    Wq = np.asarray(Wq, np.float32)
    Wk = np.asarray(Wk, np.float32)
    Wv = np.asarray(Wv, np.float32)
    Wo = np.asarray(Wo, np.float32)
    bq = np.asarray(bq, np.float32)
    bk = np.asarray(bk, np.float32)
    bv = np.asarray(bv, np.float32)
    bo = np.asarray(bo, np.float32)

    mrow = [mask[b, 0, 0] != 0 for b in range(B)]
    perms = [np.argsort(~mrow[b], kind="stable") for b in range(B)]
    nkeep = [int(mrow[b].sum()) for b in range(B)]
    tb = [min(8, max(1, math.ceil(n / 128))) for n in nkeep]
    tmax = max(tb)
    # overflow (batch, pair) units if base = tmax-1; one extra slot per
    # core when the total fits in 8, else fall back to the uniform max.
    ov = [(b, p) for b in range(B) if tb[b] == tmax for p in range(6)]
    if tmax > 1 and 0 < len(ov) <= NCORES and min(tb) < tmax:
        bkt, has_extra = tmax - 1, True
    else:
        bkt, has_extra = tmax, False
        ov = []
    KMAX = 128 * bkt

    cvec = bv @ Wo + bo
    has_cvec = bool(np.any(cvec))

    wq_p, wk_p, bq_p, bk_p = [], [], [], []
    for g in range(2):
        cs = slice(g * GW, (g + 1) * GW)
        wq_p.append(_pack6((Wq[:, cs] * 0.125).astype(f16)))
        wk_p.append(_pack6(Wk[:, cs].astype(f16)))
        bq_p.append((bq[cs] * 0.125).reshape(3, 128).T)   # [128,3]
        bk_p.append(bk[cs].reshape(3, 128).T)
    wvp = _pack6((Wv @ Wo).astype(f16))
    wvo6 = cvec.astype(f16).reshape(1, 768)

    xp_b, mk_b = [], []
    for b in range(B):
        xp_b.append(_pack6(x[b][perms[b]].T.astype(f16)))
        mk = np.full(KMAX, -1e9, np.float32)
        mk[:min(nkeep[b], KMAX)] = 0.0
        mk_b.append(mk.reshape(bkt, 128).T)

    # extra-slot assignment: overflow units go to non-owner cores first
    extras = [None] * NCORES
    if has_extra:
        order = sorted(range(NCORES), key=lambda c: c // 2 in
                       {b for b, _ in ov})
        for slot, unit in zip(order, ov):
            extras[slot] = unit

    def xe_tile(b):
        """Packed [128, 6*128] of the extra key-tile tokens of batch b."""
        lo = bkt * 128
        xs = np.zeros((128, 768), np.float32)
        hi = min(1024, lo + 128)
        xs[:hi - lo] = x[b][perms[b][lo:hi]]
        return _pack6(np.ascontiguousarray(xs.T).astype(f16))

    nsv = 6 + bkt + (3 if has_extra else 0)
    in_maps = []
    for c in range(NCORES):
        b, g = c // 2, c % 2
        sv = np.zeros((128, nsv), np.float32)
        sv[:, 0:3] = bq_p[g]
        sv[:, 3:6] = bk_p[g]
        sv[:, 6:6 + bkt] = mk_b[b]
        im = {"xp": xp_b[b], "wqp": wq_p[g], "wkp": wk_p[g], "wvp": wvp}
        if has_cvec:
            im["wvo6"] = wvo6
        if has_extra:
            if extras[c] is not None:
                eb, ep = extras[c]
                mke = np.full(128, -1e9, np.float32)
                ner = nkeep[eb] - bkt * 128
                mke[:max(0, min(128, ner))] = 0.0
                sv[:, 6 + bkt] = mke
                sv[:, 7 + bkt] = bq[ep * 128:(ep + 1) * 128] * 0.125
                sv[:, 8 + bkt] = bk[ep * 128:(ep + 1) * 128]
                im["xe"] = xp_b[eb]
                im["xekt"] = xe_tile(eb)
                im["wqe"] = _pair_cols(Wq, ep, 0.125)
                im["wke"] = _pair_cols(Wk, ep)
            else:
                sv[:, 6 + bkt] = -1e9   # dummy: fully masked
                im["xe"] = xp_b[b]
                im["xekt"] = xe_tile(b)
                im["wqe"] = _pair_cols(Wq, 0, 0.125)
                im["wke"] = _pair_cols(Wk, 0)
        im["sv"] = sv
        in_maps.append(im)
    return bkt, has_extra, has_cvec, in_maps, perms, extras


def gather_output(results, perms, extras):
    num = np.zeros((B, NH, S, 768), np.float32)
    sig = np.zeros((B, NH, S, 1), np.float32)
    def unshuffle(o):
        # [.., hh, qc, p, mq, f] -> [.., hh, q, f] with q = qc*512+mq*128+p
        o = o.transpose(0, 1, 2, 4, 3, 5)        # [j, hh, qc, mq, p, f]
        return o.reshape(o.shape[0], 2, 1024, 770)

    for c in range(NCORES):
        b, g = c // 2, c % 2
        o = unshuffle(np.asarray(results[c]["out"], np.float32))
        for j in range(3):
            for hh in range(2):
                h = g * 6 + j * 2 + hh
                num[b, h] += o[j, hh, :, :768]
                sig[b, h, :, 0] += o[j, hh, :, 768]
        if extras[c] is not None:
            eb, ep = extras[c]
            oe = unshuffle(np.asarray(
                results[c]["oute"], np.float32)[None])[0]
            for hh in range(2):
                h = ep * 2 + hh
                num[eb, h] += oe[hh, :, :768]
                sig[eb, h, :, 0] += oe[hh, :, 768]
    res = num / sig                                    # [B,NH,S,H]
    out = np.empty((B, S * NH, H), np.float32)
    ov = out.reshape(B, S, NH, H)
    for b in range(B):
        ov[b, perms[b]] = res[b].transpose(1, 0, 2)
    return out


def kernel(**inputs):
    from concourse.bass_utils import run_bass_kernel_spmd

    bkt, has_extra, has_cvec, in_maps, perms, extras = prep(**inputs)
    nc = get_program(bkt, has_extra, has_cvec)
    res = run_bass_kernel_spmd(nc, in_maps, core_ids=list(range(NCORES)))
    return gather_output(res.results, perms, extras)


if __name__ == "__main__":
    rng = np.random.default_rng(0)
    demo = {
        "x": rng.standard_normal((B, S, H), dtype=np.float32),
        "mask": rng.integers(0, 2, (B, 1, 1, S)).astype(np.int32),
        "Wq": rng.standard_normal((H, H), dtype=np.float32) / np.sqrt(H),
        "bq": np.zeros(H, np.float32),
        "Wk": rng.standard_normal((H, H), dtype=np.float32) / np.sqrt(H),
        "bk": np.zeros(H, np.float32),
        "Wv": rng.standard_normal((H, H), dtype=np.float32) / np.sqrt(H),
        "bv": np.zeros(H, np.float32),
        "Wo": rng.standard_normal((H, H), dtype=np.float32) / np.sqrt(H),
        "bo": np.zeros(H, np.float32),
    }
    out = kernel(**demo)
    print("kernel ran, output shape", out.shape)



# revision 17
# speedup vs baseline: 1.1842x; 1.1126x over previous
"""Trainium2 Bass kernel for the quirky MultiHeadAttention module.

Reference computation (B=4, S=1024, H=768, NH=12, HS=64):
    Q = (x@Wq+bq)  split into heads     [B,12,S,64]
    K = (x@Wk+bk)  split into heads     [B,12,S,64]
    V = x@Wv+bv    NOT split            [B,S,768]
    A = softmax(QK^T/8 + mask)          [B,12,S,S]
    out = (A @ V) reshaped [B, S*12, H] @ Wo + bo    -> [4, 12288, 768]

Algebraic restructuring:
  * (A @ V) @ Wo = A @ (x @ (Wv@Wo)) (+ cvec = bv@Wo + bo, realized by a
    constant row added to VW -- softmax rows sum to one).
  * Masked keys produce exp(-1e9+s) == 0 exactly, so they are dropped on
    the host and the key axis is compacted per batch.
  * The softmax denominator comes from a ones-column appended to VW.
    The device emits UNNORMALIZED numerators + the sigma column in f16;
    the host performs the division (allows partial-sum sharding).
  * The device covers at most bkt=4 key tiles (512 compacted keys) per
    batch; the few overflow keys of a denser batch (here 17) contribute
    their partial numerator/sigma on the HOST (u_e @ (x_e@Wv@Wo), exact
    f32) -- this deletes the entire device-side "extra unit" machinery
    (~10us/core of duplicated projections + a 3 MB extra output).

Sharding: 8 cores = 4 batches x 2 head-groups (6 heads each), pure SPMD.

Layouts (all transposed so no on-device transposes are needed):
    QT/KT: [feat, tok] with head pairs packed 64+64 in partitions; the
        64-row score matmuls run 2-head-concurrent via PE row groups.
    S^T = KT.T @ QT -> [k, q]  (k on partitions => mask is a per-partition
        bias folded into the Exp activation)
    U = exp(S^T) [k, q] f16 -> exactly the layout the PV matmul needs
    out = U.T @ [VW | 1] -> [q, 769] with col 768 = sigma, stored f16.
Matmul operands are f16; accumulation is f32 in PSUM.  Inputs stream
over all three DMA queues in consumption order; a burst of junk matmuls
pre-warms the PE clock gate (HAM) to 2.4 GHz before real work.  Chunk 0's
scores+exps run inside phase A (psA's qk banks) so PV starts the moment
phase B opens.  PSUM drains go 2:1 vector:scalar; output DMAs are one
per (chunk, head-half) on the sync/gpsimd queues, except the last chunk
which drains 1:1 vector/scalar and DMAs per mq-pair on all three queues
for a short tail.
"""

import math

import numpy as np

B, S, H, NH, HS = 4, 1024, 768, 12, 64
GW = 384          # head-group width = 6 heads * 64
NCORES = 8
BKT_CAP = 4       # device covers at most 4 key tiles; rest goes to host

_PROGRAM_CACHE = {}


def _pack6(a):
    """[768, N] -> partition-major [128, 6*N] (tile i at cols i*N:(i+1)*N)."""
    n = a.shape[1]
    return np.ascontiguousarray(
        a.reshape(6, 128, n).transpose(1, 0, 2).reshape(128, 6 * n))


def _build_program(bkt, has_cvec):
    """bkt: number of 128-wide key tiles per core (1..4).
    has_cvec: include the rank-1 (bv@Wo + bo) constant row in VW."""
    import concourse.mybir as mybir
    import concourse.tile as tile
    from concourse import bacc
    from concourse.bass import ds, ts

    f32 = mybir.dt.float32
    f16 = mybir.dt.float16
    AF = mybir.ActivationFunctionType

    KMAX = 128 * bkt
    if KMAX <= 512:
        kchunks = [(0, KMAX)]
    else:
        w1 = 128 * ((bkt + 1) // 2)
        kchunks = [(0, w1), (w1, KMAX - w1)]
    # sv columns: bq(3) bk(3) mk(bkt)
    nsv = 6 + bkt

    nc = bacc.Bacc(None, target_bir_lowering=False, debug=False)

    xp_d = nc.dram_tensor("xp", (128, 6 * 1024), f16, kind="ExternalInput")
    wqp_d = nc.dram_tensor("wqp", (128, 6 * 384), f16, kind="ExternalInput")
    wkp_d = nc.dram_tensor("wkp", (128, 6 * 384), f16, kind="ExternalInput")
    wvp_d = nc.dram_tensor("wvp", (128, 6 * 768), f16, kind="ExternalInput")
    sv_d = nc.dram_tensor("sv", (128, nsv), f32, kind="ExternalInput")
    if has_cvec:
        wvo6_d = nc.dram_tensor("wvo6", (1, 768), f16, kind="ExternalInput")
    # layout [j, hh, qc, p, mq, f]: DMA src iterates (partition, mq-block,
    # f), so those must be the three minor dims of the destination
    out_d = nc.dram_tensor("out", (3, 2, 2, 128, 4, 770), f16,
                           kind="ExternalOutput")

    with tile.TileContext(nc) as tc:
        with (
            tc.tile_pool(name="persist", bufs=1) as pp,
            tc.tile_pool(name="ut", bufs=4 * max(bkt, 2)) as utp,
            tc.tile_pool(name="osb", bufs=4) as op_,
        ):
            # ---- stream inputs (order within each queue = priority) ----
            sv = pp.tile([128, nsv], f32, name="sv", tag="sv")
            nc.sync.dma_start(sv[:], sv_d[:])
            bq_t = [sv[:, j:j + 1] for j in range(3)]
            bk_t = [sv[:, 3 + j:4 + j] for j in range(3)]
            mk_t = [sv[:, 6 + k:7 + k] for k in range(bkt)]

            xbig = pp.tile([128, 6 * 1024], f16, name="xbig", tag="xbig")
            wqbig = pp.tile([128, 6 * 384], f16, name="wqbig", tag="wqbig")
            wkbig = pp.tile([128, 6 * 384], f16, name="wkbig", tag="wkbig")
            wvbig = pp.tile([128, 6 * 768], f16, name="wvbig", tag="wvbig")
            if has_cvec:
                ones1 = pp.tile([1, 128], f16, name="ones1", tag="ones1")
                wvo6 = pp.tile([1, 768], f16, name="wvo6", tag="wvo6")

            def wq_piece(i):
                return (wqbig[:, i * 384:(i + 1) * 384],
                        wqp_d[:, i * 384:(i + 1) * 384])

            def wk_piece(i):
                return (wkbig[:, i * 384:(i + 1) * 384],
                        wkp_d[:, i * 384:(i + 1) * 384])

            def x_piece(i):
                return (xbig[:, i * 1024:(i + 1) * 1024],
                        xp_d[:, i * 1024:(i + 1) * 1024])

            # HW queues (sync/scalar) carry wk first (KT must never
            # stall) then one (wq, x) pair each; the fast SW queue
            # (gpsimd) carries the other four (wq, x) pairs, then wvp.
            # The QT loop consumes kt in arrival order (0,1,2,5 then 3,4).
            for i in (0, 1, 2):
                nc.sync.dma_start(*wk_piece(i))
                nc.scalar.dma_start(*wk_piece(3 + i))
            for i in (0, 1, 2):
                nc.gpsimd.dma_start(*wq_piece(i))
                nc.gpsimd.dma_start(*x_piece(i))
            nc.gpsimd.dma_start(*wq_piece(5))
            nc.gpsimd.dma_start(*x_piece(5))
            nc.sync.dma_start(*wq_piece(3))
            nc.sync.dma_start(*x_piece(3))
            nc.scalar.dma_start(*wq_piece(4))
            nc.scalar.dma_start(*x_piece(4))
            if has_cvec:
                nc.vector.memset(ones1[:], 1.0)
                nc.scalar.dma_start(wvo6[:], wvo6_d[:])
            for i in range(6):
                nc.gpsimd.dma_start(wvbig[:, i * 768:(i + 1) * 768],
                                    wvp_d[:, i * 768:(i + 1) * 768])

            xt = [xbig[:, i * 1024:(i + 1) * 1024] for i in range(6)]
            wq_t = [wqbig[:, i * 384:(i + 1) * 384] for i in range(6)]
            xkt = [xbig[:, i * 1024:i * 1024 + KMAX] for i in range(6)]
            wk_t = [wkbig[:, i * 384:(i + 1) * 384] for i in range(6)]
            wvo_t = [wvbig[:, i * 768:(i + 1) * 768] for i in range(6)]

            # persistent intermediates
            QT = [pp.tile([128, 1024], f16, name=f"QT{j}", tag=f"QT{j}")
                  for j in range(3)]
            KT = [pp.tile([128, KMAX], f16, name=f"KT{j}", tag=f"KT{j}")
                  for j in range(3)]
            VW = [pp.tile([128, 770], f16, name=f"VW{m}", tag=f"VW{m}")
                  for m in range(bkt)]

            def emit_scores(ch, sp):
                """Score MMs kt-major (2-head row-group concurrency),
                then exps hh-major so the first PV group unblocks
                after only nkt exps. Fills ch['ut']. sp: the PSUM pool
                to draw score tiles from."""
                nkt = len(ch["kt_sb"])
                qt_sb, qch, masks = ch["qt"], ch["qch"], ch["masks"]
                psS = [[None] * nkt for _ in range(2)]
                for i in range(nkt):
                    ktile, csel, _vw = ch["kt_sb"][i]
                    for hh in range(2):
                        p0 = hh * 64
                        ps = sp.tile([128, 512], f32, name="psS",
                                     tag="psS")
                        nc.tensor.matmul(
                            ps[:], ktile[p0:p0 + 64, csel],
                            qt_sb[p0:p0 + 64, qch])
                        psS[hh][i] = ps
                ut = [[None] * nkt for _ in range(2)]
                # hh-major exps let the first PV group start after
                # only nkt exps; beyond 4 tiles that ordering inverts
                # the psS buffer-recycle dependencies against the
                # in-order queues (deadlock), so fall back to the MM
                # allocation order (kt-major)
                if nkt <= 4:
                    order = [(hh, i) for hh in range(2)
                             for i in range(nkt)]
                else:
                    order = [(hh, i) for i in range(nkt)
                             for hh in range(2)]
                for hh, i in order:
                    u = utp.tile([128, 512], f16, name="ut", tag="ut")
                    nc.scalar.activation(
                        u[:], psS[hh][i][:], AF.Exp, bias=masks[i])
                    ut[hh][i] = u
                ch["ut"] = ut

            base_rings = (nc.sync, nc.gpsimd)
            chunks = []
            for ci, (j, qc) in enumerate(
                    (j, qc) for j in range(3) for qc in range(2)):
                def odst(mp, hh, j=j, qc=qc):
                    return out_d[j, hh, qc, :, ds(mp * 2, 2), :]
                def odstf(hh, j=j, qc=qc):
                    return out_d[j, hh, qc, :, :, :]
                chunks.append(dict(
                    qch=ds(qc * 512, 512),
                    kt_sb=[(KT[j], ts(kt, 128), VW[kt])
                           for kt in range(bkt)],
                    qt=QT[j], masks=mk_t, odst=odst, odstf=odstf,
                    last=False, rings=base_rings))
            # last chunk: finer DMA granularity over all 3 queues so the
            # post-final-matmul tail is short
            chunks[-1]["rings"] = (nc.sync, nc.gpsimd, nc.scalar)
            chunks[-1]["last"] = True

            # ---- phase A: projections ----
            # PE warm-up: small junk matmuls keep the tensor engine busy
            # through the HAM SHORT window (~3.4us) so real work runs at
            # 2.4 GHz from the start (count tuned to end ~when the first
            # x/wq pieces land).
            wsrc = pp.tile([128, 384], f16, name="wsrc", tag="wsrc")
            nc.vector.memset(wsrc[:], 0.0)
            with tc.tile_pool(name="psA", bufs=6, space="PSUM") as psA:
                # warmup junk shares the "vw" tag's banks -- no separate
                # pool, so no pool-close drain between warmup and QT
                for _ in range(30):
                    psw = psA.tile([64, 128], f32, name="warm", tag="vw",
                                   bufs=2)
                    nc.tensor.matmul(psw[:], wsrc[:, 0:64], wsrc[:, 0:128])
                # QT is kt-major: all six (j,qc) PSUM groups accumulate in
                # parallel so each arriving x tile is consumed immediately.
                qgroups = [(j, qc) for j in range(3) for qc in range(2)]
                qps = [psA.tile([128, 512], f32, name=f"qtp{j}{qc}", tag="qk")
                       for j, qc in qgroups]
                kt_order = (0, 1, 2, 5, 3, 4)   # input arrival order
                for ki, kt in enumerate(kt_order):
                    for gi, (j, qc) in enumerate(qgroups):
                        nc.tensor.matmul(
                            qps[gi][:], wq_t[kt][:, ts(j, 128)],
                            xt[kt][:, ds(qc * 512, 512)],
                            start=(ki == 0), stop=(ki == 5))
                for gi, (j, qc) in enumerate(qgroups):
                    nc.scalar.activation(
                        QT[j][:, ds(qc * 512, 512)], qps[gi][:], AF.Identity,
                        bias=bq_t[j])

                def emit_vw(dst, src_tiles, msel):
                    # dst[k,f] accumulates src.T @ (Wv@Wo) for one key
                    # tile; cols 768:770 are BOTH ones (the PV chains
                    # split 385+385, each f-half carrying a sigma column)
                    for fo, w in ((0, 385), (385, 383)):
                        fch = ds(fo, w)
                        ps = psA.tile([128, 385], f32, name="vw", tag="vw",
                                      bufs=2)
                        for kt in range(6):
                            nc.tensor.matmul(
                                ps[:, 0:w], src_tiles[kt][:, msel],
                                wvo_t[kt][:, fch],
                                start=(kt == 0),
                                stop=(kt == 5 and not has_cvec))
                        if has_cvec:
                            nc.tensor.matmul(
                                ps[:, 0:w], ones1[:], wvo6[:, fch],
                                start=False, stop=True)
                        nc.vector.tensor_copy(dst[:, fch], ps[:, 0:w])
                    nc.vector.memset(dst[:, 768:770], 1.0)

                def emit_kt(j):
                    for o, w in kchunks:
                        kch = ds(o, w)
                        ps2 = psA.tile([128, 512], f32, name="ktp", tag="qk")
                        for kt in range(6):
                            nc.tensor.matmul(
                                ps2[:, 0:w], wk_t[kt][:, ts(j, 128)],
                                xkt[kt][:, kch],
                                start=(kt == 0), stop=(kt == 5))
                        nc.scalar.activation(
                            KT[j][:, kch], ps2[:, 0:w], AF.Identity,
                            bias=bk_t[j])

                class _QkPool:
                    """Adapter: chunk 0's score tiles draw from psA's qk
                    ring so scores+exps run inside phase A (the exps
                    overlap the VW projections on the scalar engine and
                    PV can start the moment phase B's pools open)."""
                    def tile(self, shape, dt, name, tag):
                        return psA.tile(shape, dt, name=name, tag="qk")

                emit_kt(0)
                emit_scores(chunks[0], _QkPool())
                emit_kt(1)
                emit_kt(2)
                # KT first (wk arrives early on the HW queues), then VW
                # (wvp lands on the SW queue by ~14us).
                for m in range(bkt):
                    emit_vw(VW[m], xkt, ts(m, 128))

            # ---- phase B: attention ----
            ring_i = [0]

            def out_dma(dst, src, rings):
                rings[ring_i[0] % len(rings)].dma_start(dst, src)
                ring_i[0] += 1

            drain_i = [0]
            zb = pp.tile([128, 1], f32, name="zb", tag="zb")
            nc.gpsimd.memset(zb[:], 0.0)

            def drain(dst, src):
                # PSUM->SBUF drains 2:1 vector:scalar -- keeps the vector
                # queue short so psO banks recycle without stalling PV
                if drain_i[0] % 3 < 2:
                    nc.vector.tensor_copy(dst, src)
                else:
                    nc.scalar.activation(dst, src, AF.Identity, bias=zb[:])
                drain_i[0] += 1

            with (
                tc.tile_pool(name="psS", bufs=4, space="PSUM") as psSp,
                tc.tile_pool(name="psO", bufs=2, space="PSUM") as psOp,
            ):

                def emit_pv(ch, nxt):
                    """PV groups hh-major; each group accumulates into one
                    2-bank PSUM tile (pa bank 0, pb bank-aligned at col
                    512), drained by ONE strided cast; output DMAs go ONE
                    per (hh) (per mq-pair on the last chunk for a short
                    tail). The NEXT chunk's scores+exps are hoisted in
                    between the hh halves so its exps complete during this
                    chunk's PV (a chunk-boundary exp wait re-throttles the
                    PE clock)."""
                    nkt = len(ch["kt_sb"])
                    ut, rings = ch["ut"], ch["rings"]
                    for hh in range(2):
                        oh = op_.tile([128, 4, 770], f16, name="ob",
                                      tag="ob")
                        for mq in range(4):
                            po = psOp.tile([128, 2, 512], f32, name="psO",
                                           tag="psO")
                            pa = po[:, 0, 0:385]
                            pb = po[:, 1, 0:385]
                            for i in range(nkt):
                                nc.tensor.matmul(
                                    pb, ut[hh][i][:, ts(mq, 128)],
                                    ch["kt_sb"][i][2][:, 385:770],
                                    start=(i == 0), stop=(i == nkt - 1))
                            for i in range(nkt):
                                nc.tensor.matmul(
                                    pa, ut[hh][i][:, ts(mq, 128)],
                                    ch["kt_sb"][i][2][:, 0:385],
                                    start=(i == 0), stop=(i == nkt - 1))
                            if ch["last"]:
                                # strict 1:1 vector/scalar for a 2-lane
                                # parallel tail
                                (nc.vector.tensor_copy(oh[:, mq, :],
                                                       po[:, :, 0:385])
                                 if mq % 2 == 0 else
                                 nc.scalar.activation(oh[:, mq, :],
                                                      po[:, :, 0:385],
                                                      AF.Identity,
                                                      bias=zb[:]))
                            else:
                                drain(oh[:, mq, :], po[:, :, 0:385])
                            if ch["last"] and mq % 2 == 1:
                                out_dma(ch["odst"](mq // 2, hh),
                                        oh[:, ds(mq - 1, 2), :], rings)
                        if not ch["last"]:
                            out_dma(ch["odstf"](hh), oh[:], rings)
                        if hh == 0 and nxt is not None:
                            emit_scores(nxt, psSp)

                for ci, ch in enumerate(chunks):
                    emit_pv(ch, chunks[ci + 1] if ci + 1 < len(chunks)
                            else None)
    nc.compile()
    return nc


def get_program(bkt, has_cvec):
    key = (bkt, has_cvec)
    if key not in _PROGRAM_CACHE:
        _PROGRAM_CACHE[key] = _build_program(*key)
    return _PROGRAM_CACHE[key]


def prep(x, mask, Wq, bq, Wk, bk, Wv, bv, Wo, bo):
    """Host-side sharding/compaction. Returns (bkt, has_cvec, in_maps,
    perms, host_ctx); host_ctx carries what gather_output needs to add
    the host-side overflow-key partial sums."""
    f16 = np.float16
    x = np.asarray(x, np.float32)
    mask = np.asarray(mask)
    Wq = np.asarray(Wq, np.float32)
    Wk = np.asarray(Wk, np.float32)
    Wv = np.asarray(Wv, np.float32)
    Wo = np.asarray(Wo, np.float32)
    bq = np.asarray(bq, np.float32)
    bk = np.asarray(bk, np.float32)
    bv = np.asarray(bv, np.float32)
    bo = np.asarray(bo, np.float32)

    mrow = [mask[b, 0, 0] != 0 for b in range(B)]
    perms = [np.argsort(~mrow[b], kind="stable") for b in range(B)]
    nkeep = [int(mrow[b].sum()) for b in range(B)]
    tb = [min(8, max(1, math.ceil(n / 128))) for n in nkeep]
    bkt = min(max(tb), BKT_CAP)
    KMAX = 128 * bkt

    cvec = bv @ Wo + bo
    has_cvec = bool(np.any(cvec))
    WvWo = Wv @ Wo

    wq_p, wk_p, bq_p, bk_p = [], [], [], []
    for g in range(2):
        cs = slice(g * GW, (g + 1) * GW)
        wq_p.append(_pack6((Wq[:, cs] * 0.125).astype(f16)))
        wk_p.append(_pack6(Wk[:, cs].astype(f16)))
        bq_p.append((bq[cs] * 0.125).reshape(3, 128).T)   # [128,3]
        bk_p.append(bk[cs].reshape(3, 128).T)
    wvp = _pack6(WvWo.astype(f16))
    wvo6 = cvec.astype(f16).reshape(1, 768)

    xp_b, mk_b = [], []
    for b in range(B):
        xp_b.append(_pack6(x[b][perms[b]].T.astype(f16)))
        mk = np.full(KMAX, -1e9, np.float32)
        mk[:min(nkeep[b], KMAX)] = 0.0
        mk_b.append(mk.reshape(bkt, 128).T)

    in_maps = []
    for c in range(NCORES):
        b, g = c // 2, c % 2
        sv = np.zeros((128, 6 + bkt), np.float32)
        sv[:, 0:3] = bq_p[g]
        sv[:, 3:6] = bk_p[g]
        sv[:, 6:6 + bkt] = mk_b[b]
        im = {"xp": xp_b[b], "wqp": wq_p[g], "wkp": wk_p[g], "wvp": wvp,
              "sv": sv}
        if has_cvec:
            im["wvo6"] = wvo6
        in_maps.append(im)

    # host-side overflow keys (compacted indices beyond KMAX)
    ov = []
    for b in range(B):
        if nkeep[b] > KMAX:
            ov.append((b, x[b][perms[b][KMAX:nkeep[b]]]))
    host_ctx = dict(x=x, Wq=Wq, bq=bq, Wk=Wk, bk=bk, WvWo=WvWo,
                    cvec=cvec, ov=ov)
    return bkt, has_cvec, in_maps, perms, host_ctx


def gather_output(results, perms, host_ctx):
    num = np.zeros((B, NH, S, 768), np.float32)
    sig = np.zeros((B, NH, S, 1), np.float32)
    def unshuffle(o):
        # [.., hh, qc, p, mq, f] -> [.., hh, q, f] with q = qc*512+mq*128+p
        o = o.transpose(0, 1, 2, 4, 3, 5)        # [j, hh, qc, mq, p, f]
        return o.reshape(o.shape[0], 2, 1024, 770)

    for c in range(NCORES):
        b, g = c // 2, c % 2
        o = unshuffle(np.asarray(results[c]["out"], np.float32))
        for j in range(3):
            for hh in range(2):
                h = g * 6 + j * 2 + hh
                num[b, h] += o[j, hh, :, :768]
                sig[b, h, :, 0] += o[j, hh, :, 768]

    # host partial sums for overflow keys (exact f32)
    if host_ctx["ov"]:
        x, Wq, bq = host_ctx["x"], host_ctx["Wq"], host_ctx["bq"]
        Wk, bk = host_ctx["Wk"], host_ctx["bk"]
        WvWo, cvec = host_ctx["WvWo"], host_ctx["cvec"]
        for b, xe in host_ctx["ov"]:
            # device q-axis order == permuted token order
            Q = x[b][perms[b]] @ Wq + bq       # [S, 768]
            Ke = xe @ Wk + bk                  # [ne, 768]
            Ve = xe @ WvWo                     # [ne, 768] (+cvec via sig)
            Qh = Q.reshape(S, NH, HS)
            Kh = Ke.reshape(-1, NH, HS)
            se = np.einsum('qhd,khd->hqk', Qh, Kh) / np.sqrt(
                np.float32(HS))
            ue = np.exp(se)                    # [NH, S, ne]
            num[b] += ue @ Ve + ue.sum(-1, keepdims=True) * cvec
            sig[b, :, :, 0] += ue.sum(-1)

    res = num / sig                                    # [B,NH,S,H]
    out = np.empty((B, S * NH, H), np.float32)
    ov = out.reshape(B, S, NH, H)
    for b in range(B):
        ov[b, perms[b]] = res[b].transpose(1, 0, 2)
    return out


def kernel(**inputs):
    from concourse.bass_utils import run_bass_kernel_spmd

    bkt, has_cvec, in_maps, perms, host_ctx = prep(**inputs)
    nc = get_program(bkt, has_cvec)
    res = run_bass_kernel_spmd(nc, in_maps, core_ids=list(range(NCORES)))
    return gather_output(res.results, perms, host_ctx)


if __name__ == "__main__":
    rng = np.random.default_rng(0)
    demo = {
        "x": rng.standard_normal((B, S, H), dtype=np.float32),
        "mask": rng.integers(0, 2, (B, 1, 1, S)).astype(np.int32),
        "Wq": rng.standard_normal((H, H), dtype=np.float32) / np.sqrt(H),
        "bq": np.zeros(H, np.float32),
        "Wk": rng.standard_normal((H, H), dtype=np.float32) / np.sqrt(H),
        "bk": np.zeros(H, np.float32),
        "Wv": rng.standard_normal((H, H), dtype=np.float32) / np.sqrt(H),
        "bv": np.zeros(H, np.float32),
        "Wo": rng.standard_normal((H, H), dtype=np.float32) / np.sqrt(H),
        "bo": np.zeros(H, np.float32),
    }
    out = kernel(**demo)
    print("kernel ran, output shape", out.shape)


# revision 20
# speedup vs baseline: 1.1955x; 1.0095x over previous
"""Trainium2 Bass kernel for the quirky MultiHeadAttention module.

Reference computation (B=4, S=1024, H=768, NH=12, HS=64):
    Q = (x@Wq+bq)  split into heads     [B,12,S,64]
    K = (x@Wk+bk)  split into heads     [B,12,S,64]
    V = x@Wv+bv    NOT split            [B,S,768]
    A = softmax(QK^T/8 + mask)          [B,12,S,S]
    out = (A @ V) reshaped [B, S*12, H] @ Wo + bo    -> [4, 12288, 768]

Algebraic restructuring:
  * (A @ V) @ Wo = A @ (x @ (Wv@Wo)) (+ cvec = bv@Wo + bo, realized by a
    constant row added to VW -- softmax rows sum to one).
  * Masked keys produce exp(-1e9+s) == 0 exactly, so they are dropped on
    the host and the key axis is compacted per batch.
  * The softmax denominator comes from a ones-column appended to VW.
    The device emits UNNORMALIZED numerators + the sigma column in f16;
    the host performs the division (allows partial-sum sharding).
  * The device covers at most bkt=4 key tiles (512 compacted keys) per
    batch; the few overflow keys of a denser batch (here 17) contribute
    their partial numerator/sigma on the HOST (u_e @ (x_e@Wv@Wo), exact
    f32) -- this deletes the entire device-side "extra unit" machinery
    (~10us/core of duplicated projections + a 3 MB extra output).

Sharding: 8 cores = 4 batches x 2 head-groups (6 heads each), pure SPMD.

Layouts (all transposed so no on-device transposes are needed):
    QT/KT: [feat, tok] with head pairs packed 64+64 in partitions; the
        64-row score matmuls run 2-head-concurrent via PE row groups.
    S^T = KT.T @ QT -> [k, q]  (k on partitions => mask is a per-partition
        bias folded into the Exp activation)
    U = exp(S^T) [k, q] f16 -> exactly the layout the PV matmul needs
    out = U.T @ [VW | 1] -> [q, 769] with col 768 = sigma, stored f16.
Matmul operands are f16; accumulation is f32 in PSUM.  Inputs stream
over all three DMA queues in consumption order; a burst of junk matmuls
pre-warms the PE clock gate (HAM) to 2.4 GHz before real work.  Chunk 0's
scores+exps run inside phase A (psA's qk banks) so PV starts the moment
phase B opens.  PSUM drains go 2:1 vector:scalar; output DMAs are one
per (chunk, head-half) on the sync/gpsimd queues, except the last chunk
which drains 1:1 vector/scalar and DMAs per mq-pair on all three queues
for a short tail.
"""

import math

import numpy as np

B, S, H, NH, HS = 4, 1024, 768, 12, 64
GW = 384          # head-group width = 6 heads * 64
NCORES = 8
BKT_CAP = 4       # device covers at most 4 key tiles; rest goes to host

_PROGRAM_CACHE = {}


def _pack6(a):
    """[768, N] -> partition-major [128, 6*N] (tile i at cols i*N:(i+1)*N)."""
    n = a.shape[1]
    return np.ascontiguousarray(
        a.reshape(6, 128, n).transpose(1, 0, 2).reshape(128, 6 * n))


def _build_program(bkt, has_cvec):
    """bkt: number of 128-wide key tiles per core (1..4).
    has_cvec: include the rank-1 (bv@Wo + bo) constant row in VW."""
    import concourse.mybir as mybir
    import concourse.tile as tile
    from concourse import bacc
    from concourse.bass import ds, ts

    f32 = mybir.dt.float32
    f16 = mybir.dt.float16
    AF = mybir.ActivationFunctionType

    KMAX = 128 * bkt
    if KMAX <= 512:
        kchunks = [(0, KMAX)]
    else:
        w1 = 128 * ((bkt + 1) // 2)
        kchunks = [(0, w1), (w1, KMAX - w1)]
    # sv columns: bq(3) bk(3) mk(bkt)
    nsv = 6 + bkt

    nc = bacc.Bacc(None, target_bir_lowering=False, debug=False)

    xp_d = nc.dram_tensor("xp", (128, 6 * 1024), f16, kind="ExternalInput")
    wqp_d = nc.dram_tensor("wqp", (128, 6 * 384), f16, kind="ExternalInput")
    wkp_d = nc.dram_tensor("wkp", (128, 6 * 384), f16, kind="ExternalInput")
    wvp_d = nc.dram_tensor("wvp", (128, 6 * 768), f16, kind="ExternalInput")
    sv_d = nc.dram_tensor("sv", (128, nsv), f32, kind="ExternalInput")
    if has_cvec:
        wvo6_d = nc.dram_tensor("wvo6", (1, 768), f16, kind="ExternalInput")
    # layout [j, hh, qc, p, mq, f]: DMA src iterates (partition, mq-block,
    # f), so those must be the three minor dims of the destination
    out_d = nc.dram_tensor("out", (3, 2, 2, 128, 4, 770), f16,
                           kind="ExternalOutput")

    with tile.TileContext(nc) as tc:
        with (
            tc.tile_pool(name="persist", bufs=1) as pp,
            tc.tile_pool(name="ut", bufs=4 * max(bkt, 2)) as utp,
            tc.tile_pool(name="osb", bufs=4) as op_,
        ):
            # ---- stream inputs (order within each queue = priority) ----
            sv = pp.tile([128, nsv], f32, name="sv", tag="sv")
            nc.sync.dma_start(sv[:], sv_d[:])
            bq_t = [sv[:, j:j + 1] for j in range(3)]
            bk_t = [sv[:, 3 + j:4 + j] for j in range(3)]
            mk_t = [sv[:, 6 + k:7 + k] for k in range(bkt)]

            xbig = pp.tile([128, 6 * 1024], f16, name="xbig", tag="xbig")
            wqbig = pp.tile([128, 6 * 384], f16, name="wqbig", tag="wqbig")
            wkbig = pp.tile([128, 6 * 384], f16, name="wkbig", tag="wkbig")
            wvbig = pp.tile([128, 6 * 768], f16, name="wvbig", tag="wvbig")
            if has_cvec:
                ones1 = pp.tile([1, 128], f16, name="ones1", tag="ones1")
                wvo6 = pp.tile([1, 768], f16, name="wvo6", tag="wvo6")

            def wq_piece(i):
                return (wqbig[:, i * 384:(i + 1) * 384],
                        wqp_d[:, i * 384:(i + 1) * 384])

            def wk_piece(i):
                return (wkbig[:, i * 384:(i + 1) * 384],
                        wkp_d[:, i * 384:(i + 1) * 384])

            def x_piece(i):
                return (xbig[:, i * 1024:(i + 1) * 1024],
                        xp_d[:, i * 1024:(i + 1) * 1024])

            # HW queues (sync/scalar) carry wk first (KT must never
            # stall) then one (wq, x) pair each; the fast SW queue
            # (gpsimd) carries the other four (wq, x) pairs, then wvp.
            # The QT loop consumes kt in arrival order (0,1,2,5 then 3,4).
            for i in (0, 1, 2):
                nc.sync.dma_start(*wk_piece(i))
                nc.scalar.dma_start(*wk_piece(3 + i))
            for i in (0, 1, 2):
                nc.gpsimd.dma_start(*wq_piece(i))
                nc.gpsimd.dma_start(*x_piece(i))
            nc.gpsimd.dma_start(*wq_piece(5))
            nc.gpsimd.dma_start(*x_piece(5))
            nc.sync.dma_start(*wq_piece(3))
            nc.sync.dma_start(*x_piece(3))
            nc.scalar.dma_start(*wq_piece(4))
            nc.scalar.dma_start(*x_piece(4))
            if has_cvec:
                nc.vector.memset(ones1[:], 1.0)
                nc.scalar.dma_start(wvo6[:], wvo6_d[:])
            for i in range(6):
                nc.gpsimd.dma_start(wvbig[:, i * 768:(i + 1) * 768],
                                    wvp_d[:, i * 768:(i + 1) * 768])

            xt = [xbig[:, i * 1024:(i + 1) * 1024] for i in range(6)]
            wq_t = [wqbig[:, i * 384:(i + 1) * 384] for i in range(6)]
            xkt = [xbig[:, i * 1024:i * 1024 + KMAX] for i in range(6)]
            wk_t = [wkbig[:, i * 384:(i + 1) * 384] for i in range(6)]
            wvo_t = [wvbig[:, i * 768:(i + 1) * 768] for i in range(6)]

            # persistent intermediates
            QT = [pp.tile([128, 1024], f16, name=f"QT{j}", tag=f"QT{j}")
                  for j in range(3)]
            KT = [pp.tile([128, KMAX], f16, name=f"KT{j}", tag=f"KT{j}")
                  for j in range(3)]
            VW = [pp.tile([128, 770], f16, name=f"VW{m}", tag=f"VW{m}")
                  for m in range(bkt)]

            def emit_scores(ch, sp):
                """Score MMs kt-major (2-head row-group concurrency),
                then exps hh-major so the first PV group unblocks
                after only nkt exps. Fills ch['ut']. sp: the PSUM pool
                to draw score tiles from."""
                nkt = len(ch["kt_sb"])
                qt_sb, qch, masks = ch["qt"], ch["qch"], ch["masks"]
                psS = [[None] * nkt for _ in range(2)]
                for i in range(nkt):
                    ktile, csel, _vw = ch["kt_sb"][i]
                    for hh in range(2):
                        p0 = hh * 64
                        ps = sp.tile([128, 512], f32, name="psS",
                                     tag="psS")
                        nc.tensor.matmul(
                            ps[:], ktile[p0:p0 + 64, csel],
                            qt_sb[p0:p0 + 64, qch])
                        psS[hh][i] = ps
                ut = [[None] * nkt for _ in range(2)]
                # hh-major exps let the first PV group start after
                # only nkt exps; beyond 4 tiles that ordering inverts
                # the psS buffer-recycle dependencies against the
                # in-order queues (deadlock), so fall back to the MM
                # allocation order (kt-major)
                if nkt <= 4:
                    order = [(hh, i) for hh in range(2)
                             for i in range(nkt)]
                else:
                    order = [(hh, i) for i in range(nkt)
                             for hh in range(2)]
                for hh, i in order:
                    u = utp.tile([128, 512], f16, name="ut", tag="ut")
                    nc.scalar.activation(
                        u[:], psS[hh][i][:], AF.Exp, bias=masks[i])
                    ut[hh][i] = u
                ch["ut"] = ut

            base_rings = (nc.sync, nc.gpsimd)
            chunks = []
            for ci, (j, qc) in enumerate(
                    (j, qc) for j in range(3) for qc in range(2)):
                def odst1(mq, hh, j=j, qc=qc):
                    return out_d[j, hh, qc, :, ds(mq, 1), :]
                def odstf(hh, j=j, qc=qc):
                    return out_d[j, hh, qc, :, :, :]
                chunks.append(dict(
                    qch=ds(qc * 512, 512),
                    kt_sb=[(KT[j], ts(kt, 128), VW[kt])
                           for kt in range(bkt)],
                    qt=QT[j], masks=mk_t, odst1=odst1, odstf=odstf,
                    last=False, rings=base_rings))
            # last chunk: finer DMA granularity over all 3 queues so the
            # post-final-matmul tail is short (scalar first: its DMA
            # queue is nearly empty by then)
            chunks[-1]["rings"] = (nc.scalar, nc.sync, nc.gpsimd)
            chunks[-1]["last"] = True

            # ---- phase A: projections ----
            # PE warm-up: small junk matmuls keep the tensor engine busy
            # through the HAM SHORT window (~3.4us) so real work runs at
            # 2.4 GHz from the start (count tuned to end ~when the first
            # x/wq pieces land).
            wsrc = pp.tile([128, 384], f16, name="wsrc", tag="wsrc")
            nc.vector.memset(wsrc[:], 0.0)
            with tc.tile_pool(name="psA", bufs=6, space="PSUM") as psA:
                # warmup junk shares the "vw" tag's banks -- no separate
                # pool, so no pool-close drain between warmup and QT
                for _ in range(30):
                    psw = psA.tile([64, 128], f32, name="warm", tag="vw",
                                   bufs=2)
                    nc.tensor.matmul(psw[:], wsrc[:, 0:64], wsrc[:, 0:128])
                # QT is kt-major: all six (j,qc) PSUM groups accumulate in
                # parallel so each arriving x tile is consumed immediately.
                qgroups = [(j, qc) for j in range(3) for qc in range(2)]
                qps = [psA.tile([128, 512], f32, name=f"qtp{j}{qc}", tag="qk")
                       for j, qc in qgroups]
                kt_order = (0, 1, 2, 5, 3, 4)   # input arrival order
                for ki, kt in enumerate(kt_order):
                    for gi, (j, qc) in enumerate(qgroups):
                        nc.tensor.matmul(
                            qps[gi][:], wq_t[kt][:, ts(j, 128)],
                            xt[kt][:, ds(qc * 512, 512)],
                            start=(ki == 0), stop=(ki == 5))
                for gi, (j, qc) in enumerate(qgroups):
                    nc.scalar.activation(
                        QT[j][:, ds(qc * 512, 512)], qps[gi][:], AF.Identity,
                        bias=bq_t[j])

                def emit_vw(dst, src_tiles, msel):
                    # dst[k,f] accumulates src.T @ (Wv@Wo) for one key
                    # tile; cols 768:770 are BOTH ones (the PV chains
                    # split 385+385, each f-half carrying a sigma column)
                    for fo, w in ((0, 385), (385, 383)):
                        fch = ds(fo, w)
                        ps = psA.tile([128, 385], f32, name="vw", tag="vw",
                                      bufs=2)
                        for kt in range(6):
                            nc.tensor.matmul(
                                ps[:, 0:w], src_tiles[kt][:, msel],
                                wvo_t[kt][:, fch],
                                start=(kt == 0),
                                stop=(kt == 5 and not has_cvec))
                        if has_cvec:
                            nc.tensor.matmul(
                                ps[:, 0:w], ones1[:], wvo6[:, fch],
                                start=False, stop=True)
                        nc.vector.tensor_copy(dst[:, fch], ps[:, 0:w])
                    nc.vector.memset(dst[:, 768:770], 1.0)

                def emit_kt(j):
                    for o, w in kchunks:
                        kch = ds(o, w)
                        ps2 = psA.tile([128, 512], f32, name="ktp", tag="qk")
                        for kt in range(6):
                            nc.tensor.matmul(
                                ps2[:, 0:w], wk_t[kt][:, ts(j, 128)],
                                xkt[kt][:, kch],
                                start=(kt == 0), stop=(kt == 5))
                        nc.scalar.activation(
                            KT[j][:, kch], ps2[:, 0:w], AF.Identity,
                            bias=bk_t[j])

                class _QkPool:
                    """Adapter: chunk 0's score tiles draw from psA's qk
                    ring so scores+exps run inside phase A (the exps
                    overlap the VW projections on the scalar engine and
                    PV can start the moment phase B's pools open)."""
                    def tile(self, shape, dt, name, tag):
                        return psA.tile(shape, dt, name=name, tag="qk")

                emit_kt(0)
                emit_scores(chunks[0], _QkPool())
                emit_kt(1)
                emit_kt(2)
                # KT first (wk arrives early on the HW queues), then VW
                # (wvp lands on the SW queue by ~14us).
                for m in range(bkt):
                    emit_vw(VW[m], xkt, ts(m, 128))

            # ---- phase B: attention ----
            ring_i = [0]

            def out_dma(dst, src, rings):
                rings[ring_i[0] % len(rings)].dma_start(dst, src)
                ring_i[0] += 1

            drain_i = [0]
            zb = pp.tile([128, 1], f32, name="zb", tag="zb")
            nc.gpsimd.memset(zb[:], 0.0)

            def drain(dst, src):
                # PSUM->SBUF drains 2:1 vector:scalar -- keeps the vector
                # queue short so psO banks recycle without stalling PV
                if drain_i[0] % 3 < 2:
                    nc.vector.tensor_copy(dst, src)
                else:
                    nc.scalar.activation(dst, src, AF.Identity, bias=zb[:])
                drain_i[0] += 1

            with (
                tc.tile_pool(name="psS", bufs=4, space="PSUM") as psSp,
                tc.tile_pool(name="psO", bufs=2, space="PSUM") as psOp,
            ):

                def emit_pv(ch, nxt):
                    """PV groups hh-major; each group accumulates into one
                    2-bank PSUM tile (pa bank 0, pb bank-aligned at col
                    512), drained by ONE strided cast; output DMAs go ONE
                    per (hh) (per mq-pair on the last chunk for a short
                    tail). The NEXT chunk's scores+exps are hoisted in
                    between the hh halves so its exps complete during this
                    chunk's PV (a chunk-boundary exp wait re-throttles the
                    PE clock)."""
                    nkt = len(ch["kt_sb"])
                    ut, rings = ch["ut"], ch["rings"]
                    for hh in range(2):
                        oh = op_.tile([128, 4, 770], f16, name="ob",
                                      tag="ob")
                        for mq in range(4):
                            po = psOp.tile([128, 2, 512], f32, name="psO",
                                           tag="psO")
                            pa = po[:, 0, 0:385]
                            pb = po[:, 1, 0:385]
                            for i in range(nkt):
                                nc.tensor.matmul(
                                    pb, ut[hh][i][:, ts(mq, 128)],
                                    ch["kt_sb"][i][2][:, 385:770],
                                    start=(i == 0), stop=(i == nkt - 1))
                            for i in range(nkt):
                                nc.tensor.matmul(
                                    pa, ut[hh][i][:, ts(mq, 128)],
                                    ch["kt_sb"][i][2][:, 0:385],
                                    start=(i == 0), stop=(i == nkt - 1))
                            if ch["last"]:
                                # strict 1:1 vector/scalar for a 2-lane
                                # parallel tail
                                (nc.vector.tensor_copy(oh[:, mq, :],
                                                       po[:, :, 0:385])
                                 if mq % 2 == 0 else
                                 nc.scalar.activation(oh[:, mq, :],
                                                      po[:, :, 0:385],
                                                      AF.Identity,
                                                      bias=zb[:]))
                            else:
                                drain(oh[:, mq, :], po[:, :, 0:385])
                            if ch["last"]:
                                out_dma(ch["odst1"](mq, hh),
                                        oh[:, ds(mq, 1), :], rings)
                        if not ch["last"]:
                            out_dma(ch["odstf"](hh), oh[:], rings)
                        if hh == 0 and nxt is not None:
                            emit_scores(nxt, psSp)

                for ci, ch in enumerate(chunks):
                    emit_pv(ch, chunks[ci + 1] if ci + 1 < len(chunks)
                            else None)
    nc.compile()
    return nc


def get_program(bkt, has_cvec):
    key = (bkt, has_cvec)
    if key not in _PROGRAM_CACHE:
        _PROGRAM_CACHE[key] = _build_program(*key)
    return _PROGRAM_CACHE[key]


def prep(x, mask, Wq, bq, Wk, bk, Wv, bv, Wo, bo):
    """Host-side sharding/compaction. Returns (bkt, has_cvec, in_maps,
    perms, host_ctx); host_ctx carries what gather_output needs to add
    the host-side overflow-key partial sums."""
    f16 = np.float16
    x = np.asarray(x, np.float32)
    mask = np.asarray(mask)
    Wq = np.asarray(Wq, np.float32)
    Wk = np.asarray(Wk, np.float32)
    Wv = np.asarray(Wv, np.float32)
    Wo = np.asarray(Wo, np.float32)
    bq = np.asarray(bq, np.float32)
    bk = np.asarray(bk, np.float32)
    bv = np.asarray(bv, np.float32)
    bo = np.asarray(bo, np.float32)

    mrow = [mask[b, 0, 0] != 0 for b in range(B)]
    perms = [np.argsort(~mrow[b], kind="stable") for b in range(B)]
    nkeep = [int(mrow[b].sum()) for b in range(B)]
    tb = [min(8, max(1, math.ceil(n / 128))) for n in nkeep]
    bkt = min(max(tb), BKT_CAP)
    KMAX = 128 * bkt

    cvec = bv @ Wo + bo
    has_cvec = bool(np.any(cvec))
    WvWo = Wv @ Wo

    wq_p, wk_p, bq_p, bk_p = [], [], [], []
    for g in range(2):
        cs = slice(g * GW, (g + 1) * GW)
        wq_p.append(_pack6((Wq[:, cs] * 0.125).astype(f16)))
        wk_p.append(_pack6(Wk[:, cs].astype(f16)))
        bq_p.append((bq[cs] * 0.125).reshape(3, 128).T)   # [128,3]
        bk_p.append(bk[cs].reshape(3, 128).T)
    wvp = _pack6(WvWo.astype(f16))
    wvo6 = cvec.astype(f16).reshape(1, 768)

    xp_b, mk_b = [], []
    for b in range(B):
        xp_b.append(_pack6(x[b][perms[b]].T.astype(f16)))
        mk = np.full(KMAX, -1e9, np.float32)
        mk[:min(nkeep[b], KMAX)] = 0.0
        mk_b.append(mk.reshape(bkt, 128).T)

    in_maps = []
    for c in range(NCORES):
        b, g = c // 2, c % 2
        sv = np.zeros((128, 6 + bkt), np.float32)
        sv[:, 0:3] = bq_p[g]
        sv[:, 3:6] = bk_p[g]
        sv[:, 6:6 + bkt] = mk_b[b]
        im = {"xp": xp_b[b], "wqp": wq_p[g], "wkp": wk_p[g], "wvp": wvp,
              "sv": sv}
        if has_cvec:
            im["wvo6"] = wvo6
        in_maps.append(im)

    # host-side overflow keys (compacted indices beyond KMAX)
    ov = []
    for b in range(B):
        if nkeep[b] > KMAX:
            ov.append((b, x[b][perms[b][KMAX:nkeep[b]]]))
    host_ctx = dict(x=x, Wq=Wq, bq=bq, Wk=Wk, bk=bk, WvWo=WvWo,
                    cvec=cvec, ov=ov)
    return bkt, has_cvec, in_maps, perms, host_ctx


def gather_output(results, perms, host_ctx):
    num = np.zeros((B, NH, S, 768), np.float32)
    sig = np.zeros((B, NH, S, 1), np.float32)
    def unshuffle(o):
        # [.., hh, qc, p, mq, f] -> [.., hh, q, f] with q = qc*512+mq*128+p
        o = o.transpose(0, 1, 2, 4, 3, 5)        # [j, hh, qc, mq, p, f]
        return o.reshape(o.shape[0], 2, 1024, 770)

    for c in range(NCORES):
        b, g = c // 2, c % 2
        o = unshuffle(np.asarray(results[c]["out"], np.float32))
        for j in range(3):
            for hh in range(2):
                h = g * 6 + j * 2 + hh
                num[b, h] += o[j, hh, :, :768]
                sig[b, h, :, 0] += o[j, hh, :, 768]

    # host partial sums for overflow keys (exact f32)
    if host_ctx["ov"]:
        x, Wq, bq = host_ctx["x"], host_ctx["Wq"], host_ctx["bq"]
        Wk, bk = host_ctx["Wk"], host_ctx["bk"]
        WvWo, cvec = host_ctx["WvWo"], host_ctx["cvec"]
        for b, xe in host_ctx["ov"]:
            # device q-axis order == permuted token order
            Q = x[b][perms[b]] @ Wq + bq       # [S, 768]
            Ke = xe @ Wk + bk                  # [ne, 768]
            Ve = xe @ WvWo                     # [ne, 768] (+cvec via sig)
            Qh = Q.reshape(S, NH, HS)
            Kh = Ke.reshape(-1, NH, HS)
            se = np.einsum('qhd,khd->hqk', Qh, Kh) / np.sqrt(
                np.float32(HS))
            ue = np.exp(se)                    # [NH, S, ne]
            num[b] += ue @ Ve + ue.sum(-1, keepdims=True) * cvec
            sig[b, :, :, 0] += ue.sum(-1)

    res = num / sig                                    # [B,NH,S,H]
    out = np.empty((B, S * NH, H), np.float32)
    ov = out.reshape(B, S, NH, H)
    for b in range(B):
        ov[b, perms[b]] = res[b].transpose(1, 0, 2)
    return out


def kernel(**inputs):
    from concourse.bass_utils import run_bass_kernel_spmd

    bkt, has_cvec, in_maps, perms, host_ctx = prep(**inputs)
    nc = get_program(bkt, has_cvec)
    res = run_bass_kernel_spmd(nc, in_maps, core_ids=list(range(NCORES)))
    return gather_output(res.results, perms, host_ctx)


if __name__ == "__main__":
    rng = np.random.default_rng(0)
    demo = {
        "x": rng.standard_normal((B, S, H), dtype=np.float32),
        "mask": rng.integers(0, 2, (B, 1, 1, S)).astype(np.int32),
        "Wq": rng.standard_normal((H, H), dtype=np.float32) / np.sqrt(H),
        "bq": np.zeros(H, np.float32),
        "Wk": rng.standard_normal((H, H), dtype=np.float32) / np.sqrt(H),
        "bk": np.zeros(H, np.float32),
        "Wv": rng.standard_normal((H, H), dtype=np.float32) / np.sqrt(H),
        "bv": np.zeros(H, np.float32),
        "Wo": rng.standard_normal((H, H), dtype=np.float32) / np.sqrt(H),
        "bo": np.zeros(H, np.float32),
    }
    out = kernel(**demo)
    print("kernel ran, output shape", out.shape)
